# revision 12
# baseline (speedup 1.0000x reference)
"""Trainium2 Bass kernel for a binarized (1w1a) ResNet BasicBlock.

  out = BN2(bconv3x3(sign(BN1(bconv3x3(sign(x), sign(w1))), g1, b1), sign(w2)), g2, b2) + x

with training-mode (sync) BatchNorm over (N, H, W) and identity shortcut.
Shapes: x [64, 256, 28, 28] f32, w [256, 256, 3, 3] f32, g/b [256] f32.

Strategy (8 NeuronCores, data-parallel over batch, 8 images/core):
  - conv3x3 = 9 shifted matmuls over a zero-padded 30x30 spatial layout.
    Activations are sign() in fp8e4 (+-1 exact); contraction over 256 input
    channels runs as a single fp8 DoubleRow matmul (K=128 partitions x 2).
    Each psum chunk computes only the 14x28 interior rows (392 cols) via a
    strided rhs AP - no wasted border columns.
  - BatchNorm needs global (sync) stats: per-chunk channel sum/sumsq are
    accumulated on the fly (DVE copy w/ accum_out + Square w/ accum_out),
    then all-reduced across the 8 cores via ncfw. Layer 1 uses ONE combined
    AllReduce for both channel blocks; layer 2 keeps per-block AllReduces so
    block 0's BN+shortcut+store overlaps block 1's conv. A dummy AllReduce
    issued at kernel start absorbs the expensive first-collective setup
    (~70us) under the conv1 window.
  - Weights are sign()ed and laid out host-side (negligible: 0.05% of FLOPs).
  - Scheduling: interlayer sign() is interleaved image-by-image with conv2
    so conv2 starts right after the first image's planes are binarized;
    block-0 BN2 coefficients and outputs are issued interleaved into the
    conv2-block-1 chunk loop so they run in its shadow; output DMA streams
    per chunk.
"""

import os
import sys

sys.path.insert(0, "/opt/trn_rl_repo")

import numpy as np
import ml_dtypes
from contextlib import ExitStack

import concourse.bass as bass
import concourse.tile as tile
from concourse import bacc, mybir
from concourse import bass_utils

N_CORES = 8
NTOT, C, H, W = 64, 256, 28, 28
NPC = NTOT // N_CORES          # images per core
P, J = 128, 2                  # partition block, channel blocks
PW = 30                        # padded width/height
IMG = PW * PW                  # 900
G = 32                         # front pad (keeps plane starts staggered)
PLANE = 1060                   # padded plane stride in the fp8 layout
HW = H * W                     # 784
COLS = 392                     # one psum chunk: 14 interior rows x 28 cols
CNT = float(NTOT * HW)         # BN reduction count: 50176
EPS = 1e-5

F32 = mybir.dt.float32
F16 = mybir.dt.float16
F8 = mybir.dt.float8e4

ADD = mybir.AluOpType.add
MULT = mybir.AluOpType.mult

_cache = {}


USE_392 = False                # strided 392-col rhs vs padded 450-col rhs
SQ20_ON_ACT = True             # conv2-cb0 squares: ACT (True) or DVE (False)
CHUNK450 = 15 * PW             # 450 padded positions per 450-col chunk


def _chunk(nc, xs, wts, craw, cb, n, half, psum, scratch, sums, sumsqs,
           sq_on_act):
    """One binary-conv psum chunk (14 interior rows) + stats accumulation."""
    if USE_392:
        acc = psum.tile([P, COLS], F32, tag="acc")
        xv = xs[:, 2 * n:2 * n + 2, G:G + IMG].rearrange(
            "p a (r c) -> p a r c", c=PW)
        for k in range(9):
            kh, kw = divmod(k, 3)
            r0 = 14 * half + kh
            nc.tensor.matmul(
                acc,
                lhsT=wts[:, k, :, cb * P:(cb + 1) * P],
                rhs=xv[:, :, r0:r0 + 14, kw:kw + 28],
                start=(k == 0),
                stop=(k == 8),
                perf_mode=mybir.MatmulPerfMode.DoubleRow,
            )
        intr = acc
    else:
        acc = psum.tile([P, CHUNK450], F32, tag="acc")
        for k in range(9):
            kh, kw = divmod(k, 3)
            base = G + (15 * half + kh - 1) * PW + (kw - 1)
            nc.tensor.matmul(
                acc,
                lhsT=wts[:, k, :, cb * P:(cb + 1) * P],
                rhs=xs[:, 2 * n:2 * n + 2, base:base + CHUNK450],
                start=(k == 0),
                stop=(k == 8),
                perf_mode=mybir.MatmulPerfMode.DoubleRow,
            )
        rows = acc.rearrange("p (r c) -> p r c", c=PW)
        r_lo = 1 - half  # skip padded row 0 in the first chunk
        intr = rows[:, r_lo:r_lo + 14, 1:1 + W]
    ci = 2 * n + half
    sl = slice(half * COLS, (half + 1) * COLS)
    # copy to f16 staging + per-chunk channel sums (DVE)
    nc.vector.tensor_scalar(
        out=craw[:, cb, n, sl], in0=intr, scalar1=0.0, scalar2=0.0,
        op0=ADD, op1=ADD, accum_out=sums[:, ci:ci + 1],
    )
    # per-chunk channel sum-of-squares
    sq = scratch.tile([P, COLS], F32, tag="sq")
    if sq_on_act:
        nc.scalar.activation(
            sq, intr, mybir.ActivationFunctionType.Square,
            accum_out=sumsqs[:, ci:ci + 1],
        )
    else:
        # DVE square of the f16 staging copy (psum can't be read twice)
        nc.vector.scalar_tensor_tensor(
            sq, in0=craw[:, cb, n, sl], scalar=1.0, in1=craw[:, cb, n, sl],
            op0=MULT, op1=MULT, accum_out=sumsqs[:, ci:ci + 1],
        )


def _bn_coeffs(nc, small, s_col, q_col, g_t, b_t, eps_t, tag):
    """Global-stat BN coefficients: scale = g*rstd, bias = b - mean*scale."""
    mean = small.tile([P, 1], F32, name=f"mean{tag}", tag=f"mean{tag}")
    nc.vector.tensor_scalar_mul(mean, s_col, 1.0 / CNT)
    ex2 = small.tile([P, 1], F32, name=f"ex2{tag}", tag=f"ex2{tag}")
    nc.vector.tensor_scalar_mul(ex2, q_col, 1.0 / CNT)
    m2 = small.tile([P, 1], F32, name=f"m2{tag}", tag=f"m2{tag}")
    nc.vector.tensor_mul(m2, mean, mean)
    var = small.tile([P, 1], F32, name=f"var{tag}", tag=f"var{tag}")
    nc.vector.tensor_sub(var, ex2, m2)
    sd = small.tile([P, 1], F32, name=f"sd{tag}", tag=f"sd{tag}")
    nc.scalar.activation(sd, var, mybir.ActivationFunctionType.Sqrt,
                         bias=eps_t)
    rstd = small.tile([P, 1], F32, name=f"rstd{tag}", tag=f"rstd{tag}")
    nc.vector.reciprocal(rstd, sd)
    scale = small.tile([P, 1], F32, name=f"scale{tag}", tag=f"scale{tag}")
    nc.vector.tensor_mul(scale, g_t, rstd)
    ms = small.tile([P, 1], F32, name=f"ms{tag}", tag=f"ms{tag}")
    nc.vector.tensor_mul(ms, mean, scale)
    bias = small.tile([P, 1], F32, name=f"bias{tag}", tag=f"bias{tag}")
    nc.vector.tensor_sub(bias, b_t, ms)
    return scale, bias


def _memset_borders(nc, xs):
    """Zero the guard bands and the 1-px padding border of every plane."""
    nc.vector.memset(xs[:, :, 0:G], 0.0)                         # low guards
    nc.vector.memset(xs[:, :, G + IMG:], 0.0)                    # high guards
    nc.vector.memset(xs[:, :, G:G + PW], 0.0)                    # top rows
    nc.vector.memset(xs[:, :, G + IMG - PW:G + IMG], 0.0)        # bottom rows
    mid = xs[:, :, G + PW:G + IMG - PW].rearrange(
        "p a (r c) -> p a r c", c=PW)
    nc.vector.memset(mid[:, :, :, 0:1], 0.0)                     # left cols
    nc.vector.memset(mid[:, :, :, PW - 1:PW], 0.0)               # right cols


def _build():
    nc = bacc.Bacc("TRN2", target_bir_lowering=False, debug=False,
                   num_devices=N_CORES)

    x_d = nc.dram_tensor("x", [NPC, C, H, W], F32, kind="ExternalInput").ap()
    w1_d = nc.dram_tensor("w1p", [P, 9, J, C], F8, kind="ExternalInput").ap()
    w2_d = nc.dram_tensor("w2p", [P, 9, J, C], F8, kind="ExternalInput").ap()
    gb1_d = nc.dram_tensor("gb1", [2, J, P], F32, kind="ExternalInput").ap()
    gb2_d = nc.dram_tensor("gb2", [2, J, P], F32, kind="ExternalInput").ap()
    y_d = nc.dram_tensor("y", [NPC, C, H, W], F32, kind="ExternalOutput").ap()

    groups = [list(range(N_CORES))]

    with tile.TileContext(nc) as tc, ExitStack() as ctx:
        big = ctx.enter_context(tc.tile_pool(name="big", bufs=1))
        small = ctx.enter_context(tc.tile_pool(name="small", bufs=1))
        psum = ctx.enter_context(tc.tile_pool(name="psum", bufs=8,
                                              space="PSUM"))
        scratch = ctx.enter_context(tc.tile_pool(name="scratch", bufs=2))
        outp = ctx.enter_context(tc.tile_pool(name="outp", bufs=6))
        dram = ctx.enter_context(tc.tile_pool(name="dram", bufs=1,
                                              space="DRAM"))

        def _ar(st, tag):
            """ncfw AllReduce of a [P, k] stats tile; returns gathered tile.

            All three steps stay on the gpsimd queue: a cross-queue wait on a
            collective's completion has no reliable hardware semaphore path
            (hangs on HW), so in-queue ordering is load-bearing here.
            """
            k = st.shape[-1]
            ar_in = dram.tile([P, k], F32, name=f"ari{tag}")
            ar_out = dram.tile([P, k], F32, name=f"aro{tag}")
            nc.gpsimd.dma_start(out=ar_in, in_=st)
            nc.gpsimd.collective_compute(
                "AllReduce", ADD, replica_groups=groups,
                ins=[ar_in.opt()], outs=[ar_out.opt()],
            )
            stg = small.tile([P, k], F32, name=f"arg{tag}", tag=f"arg{tag}")
            nc.gpsimd.dma_start(out=stg, in_=ar_out)
            return stg

        def _emit_out(cb, ci, scale, bias, style):
            """BN2 + shortcut + store for one 392-col chunk."""
            n, half = divmod(ci, 2)
            sl = slice(half * COLS, (half + 1) * COLS)
            yt = outp.tile([P, COLS], F32, tag="yt")
            if style == "dve":  # scale/bias leg on DVE
                nc.vector.tensor_scalar(
                    out=yt, in0=c2raw[:, cb, n, sl],
                    scalar1=scale, scalar2=bias, op0=MULT, op1=ADD,
                )
            else:  # scale/bias leg on the ACT engine
                nc.scalar.activation(
                    yt, c2raw[:, cb, n, sl],
                    mybir.ActivationFunctionType.Identity,
                    bias=bias, scale=scale,
                )
            yo = outp.tile([P, COLS], F32, tag="yo")
            nc.vector.tensor_add(yo, yt, xstage[:, cb, n, sl])
            nc.sync.dma_start(
                out=y_d[n, cb * P:(cb + 1) * P].rearrange(
                    "p h w -> p (h w)")[:, sl],
                in_=yo,
            )

        # ---- dummy AllReduce first: absorbs the one-time ncfw collective
        # setup (~70us) under the input-DMA/conv1 window and gives the 8
        # cores a coordinated start.
        zs = small.tile([P, 1], F32, tag="zs")
        nc.vector.memset(zs, 0.0)
        dummy_in = dram.tile([P, 1], F32)
        dummy_out = dram.tile([P, 1], F32)
        nc.sync.dma_start(out=dummy_in, in_=zs)
        nc.gpsimd.collective_compute(
            "AllReduce", ADD, replica_groups=groups,
            ins=[dummy_in.opt()], outs=[dummy_out.opt()],
        )

        # ---- padded fp8 sign planes (borders zeroed once)
        xstage = big.tile([P, J, NPC, HW], F32)
        xs1 = big.tile([P, NPC * J, PLANE], F8)
        xs2 = big.tile([P, NPC * J, PLANE], F8)
        _memset_borders(nc, xs1)
        _memset_borders(nc, xs2)
        eps_t = small.tile([P, 1], F32, tag="eps")
        nc.vector.memset(eps_t, EPS)

        # ---- x in (image-major), sign to fp8; conv1 weights right after
        # image 0 so the first matmul can start ASAP
        w1s = big.tile([P, 9, J, C], F8)
        for n in range(NPC):
            for j in range(J):
                nc.sync.dma_start(
                    out=xstage[:, j, n, :],
                    in_=x_d[n, j * P:(j + 1) * P].rearrange(
                        "p h w -> p (h w)"),
                )
                interior = xs1[:, 2 * n + j, G:G + IMG].rearrange(
                    "p (r c) -> p r c", c=PW)[:, 1:1 + H, 1:1 + W]
                nc.scalar.sign(
                    interior,
                    xstage[:, j, n, :].rearrange("p (r c) -> p r c", c=W),
                )
            if n == 0:
                nc.sync.dma_start(out=w1s, in_=w1_d)

        w2s = big.tile([P, 9, J, C], F8)
        nc.sync.dma_start(out=w2s, in_=w2_d)
        gb_t = []
        for li, gb_d in enumerate((gb1_d, gb2_d)):
            per = []
            for j in range(J):
                g_t = small.tile([P, 1], F32, name=f"g{li}{j}",
                                 tag=f"g{li}{j}")
                b_t = small.tile([P, 1], F32, name=f"b{li}{j}",
                                 tag=f"b{li}{j}")
                nc.sync.dma_start(out=g_t,
                                  in_=gb_d[0, j].rearrange("(p o) -> p o",
                                                           o=1))
                nc.sync.dma_start(out=b_t,
                                  in_=gb_d[1, j].rearrange("(p o) -> p o",
                                                           o=1))
                per.append((g_t, b_t))
            gb_t.append(per)

        # ---- layer 1: conv both channel blocks, then ONE combined AR
        c1raw = big.tile([P, J, NPC, HW], F16)
        c2raw = big.tile([P, J, NPC, HW], F16)
        st1s = []
        for cb in range(2):
            sums = small.tile([P, 16], F32, name=f"s1{cb}", tag=f"s1{cb}")
            sumsqs = small.tile([P, 16], F32, name=f"q1{cb}", tag=f"q1{cb}")
            for n in range(NPC):
                for half in range(2):
                    _chunk(nc, xs1, w1s, c1raw, cb, n, half, psum, scratch,
                           sums, sumsqs, sq_on_act=True)
            st1s.append((sums, sumsqs))
        st1 = small.tile([P, 4], F32, tag="st1")
        for cb in range(2):
            nc.vector.reduce_sum(st1[:, 2 * cb:2 * cb + 1], st1s[cb][0],
                                 axis=mybir.AxisListType.X)
            nc.vector.reduce_sum(st1[:, 2 * cb + 1:2 * cb + 2], st1s[cb][1],
                                 axis=mybir.AxisListType.X)
        stg1 = _ar(st1, "1")
        sc1 = []
        for j in range(2):
            s, b = _bn_coeffs(nc, small, stg1[:, 2 * j:2 * j + 1],
                              stg1[:, 2 * j + 1:2 * j + 2],
                              gb_t[0][j][0], gb_t[0][j][1], eps_t, f"1{j}")
            sc1.append((s, b))

        # ---- conv2 block 0, interlayer sign interleaved image-by-image
        sums20 = small.tile([P, 16], F32, tag="s20")
        sumsqs20 = small.tile([P, 16], F32, tag="q20")
        for n in range(NPC):
            for j in range(2):
                interior = xs2[:, 2 * n + j, G:G + IMG].rearrange(
                    "p (r c) -> p r c", c=PW)[:, 1:1 + H, 1:1 + W]
                nc.scalar.activation(
                    interior,
                    c1raw[:, j, n, :].rearrange("p (r c) -> p r c", c=W),
                    mybir.ActivationFunctionType.Sign,
                    bias=sc1[j][1], scale=sc1[j][0],
                )
            for half in range(2):
                _chunk(nc, xs2, w2s, c2raw, 0, n, half, psum, scratch,
                       sums20, sumsqs20, sq_on_act=SQ20_ON_ACT)
        st20 = small.tile([P, 2], F32, tag="st20")
        nc.vector.reduce_sum(st20[:, 0:1], sums20, axis=mybir.AxisListType.X)
        nc.vector.reduce_sum(st20[:, 1:2], sumsqs20,
                             axis=mybir.AxisListType.X)
        stg20 = _ar(st20, "20")

        # ---- conv2 block 1, with block-0 coeffs+outputs in its shadow
        sums21 = small.tile([P, 16], F32, tag="s21")
        sumsqs21 = small.tile([P, 16], F32, tag="q21")
        sc20 = None
        for n in range(NPC):
            for half in range(2):
                _chunk(nc, xs2, w2s, c2raw, 1, n, half, psum, scratch,
                       sums21, sumsqs21, sq_on_act=True)
                ci = 2 * n + half
                if ci == 5:
                    sc20 = _bn_coeffs(nc, small, stg20[:, 0:1], stg20[:, 1:2],
                                      gb_t[1][0][0], gb_t[1][0][1], eps_t,
                                      "20")
                if ci >= 7:
                    _emit_out(0, ci - 7, sc20[0], sc20[1], "dve")
        st21 = small.tile([P, 2], F32, tag="st21")
        nc.vector.reduce_sum(st21[:, 0:1], sums21, axis=mybir.AxisListType.X)
        nc.vector.reduce_sum(st21[:, 1:2], sumsqs21,
                             axis=mybir.AxisListType.X)
        stg21 = _ar(st21, "21")

        # leftover block-0 outputs run during the AR mesh
        for ci in range(9, 16):
            _emit_out(0, ci, sc20[0], sc20[1], "act")
        sc21 = _bn_coeffs(nc, small, stg21[:, 0:1], stg21[:, 1:2],
                          gb_t[1][1][0], gb_t[1][1][1], eps_t, "21")
        for ci in range(16):
            _emit_out(1, ci, sc21[0], sc21[1], "dve" if ci < 2 else "act")

    nc.compile()
    return nc


def _pack_w(w):
    # [co, ci, kh, kw] -> sign -> [ci%128, kh*3+kw, ci//128, co] fp8e4
    s = np.sign(w.astype(np.float32)).reshape(C, J, P, 9)
    return np.ascontiguousarray(s.transpose(2, 3, 1, 0)).astype(
        ml_dtypes.float8_e4m3)


def _pack_gb(g, b):
    return np.ascontiguousarray(
        np.stack([g, b]).astype(np.float32).reshape(2, J, P))


def kernel(x, w1, g1, b1, w2, g2, b2, _profile=False):
    if "nc" not in _cache:
        _cache["nc"] = _build()
    nc = _cache["nc"]

    x = np.ascontiguousarray(x, np.float32)
    w1p, w2p = _pack_w(w1), _pack_w(w2)
    gb1, gb2 = _pack_gb(g1, b1), _pack_gb(g2, b2)
    in_maps = [
        {"x": x[c * NPC:(c + 1) * NPC], "w1p": w1p, "w2p": w2p,
         "gb1": gb1, "gb2": gb2}
        for c in range(N_CORES)
    ]
    res = bass_utils.run_bass_kernel_spmd(
        nc, in_maps, core_ids=list(range(N_CORES)), trace=_profile)
    y = np.concatenate([res.results[c]["y"] for c in range(N_CORES)], axis=0)
    if _profile:
        kernel.last_exec_time_ns = res.exec_time_ns
        kernel.last_results = res
    return y


# revision 14
# speedup vs baseline: 1.1006x; 1.1006x over previous
"""Trainium2 Bass kernel for a binarized (1w1a) ResNet BasicBlock.

  out = BN2(bconv3x3(sign(BN1(bconv3x3(sign(x), sign(w1))), g1, b1), sign(w2)), g2, b2) + x

with training-mode (sync) BatchNorm over (N, H, W) and identity shortcut.
Shapes: x [64, 256, 28, 28] f32, w [256, 256, 3, 3] f32, g/b [256] f32.

Strategy (8 NeuronCores, data-parallel over batch, 8 images/core):
  - conv3x3 = 9 shifted matmuls over a zero-padded 30x30 spatial layout.
    Activations are sign() in fp8e4 (+-1 exact); contraction over 256 input
    channels runs as a single fp8 DoubleRow matmul (K=128 partitions x 2).
    Each psum chunk computes only the 14x28 interior rows (392 cols) via a
    strided rhs AP - no wasted border columns.
  - BatchNorm needs global (sync) stats: per-chunk channel sum/sumsq are
    accumulated on the fly (DVE copy w/ accum_out + Square w/ accum_out),
    then all-reduced across the 8 cores via ncfw. Layer 1 uses ONE combined
    AllReduce for both channel blocks; layer 2 keeps per-block AllReduces so
    block 0's BN+shortcut+store overlaps block 1's conv. A dummy AllReduce
    issued at kernel start absorbs the expensive first-collective setup
    (~70us) under the conv1 window.
  - Weights are sign()ed and laid out host-side (negligible: 0.05% of FLOPs).
  - Scheduling: interlayer sign() is interleaved image-by-image with conv2
    so conv2 starts right after the first image's planes are binarized;
    block-0 BN2 coefficients and outputs are issued interleaved into the
    conv2-block-1 chunk loop so they run in its shadow; output DMA streams
    per chunk.
"""

import os
import sys

sys.path.insert(0, "/opt/trn_rl_repo")

import numpy as np
import ml_dtypes
from contextlib import ExitStack

import concourse.bass as bass
import concourse.tile as tile
from concourse import bacc, mybir
from concourse import bass_utils

N_CORES = 8
NTOT, C, H, W = 64, 256, 28, 28
NPC = NTOT // N_CORES          # images per core
P, J = 128, 2                  # partition block, channel blocks
PW = 30                        # padded width/height
IMG = PW * PW                  # 900
G = 32                         # front pad (keeps plane starts staggered)
PLANE = 1060                   # padded plane stride in the fp8 layout
HW = H * W                     # 784
COLS = 392                     # one psum chunk: 14 interior rows x 28 cols
CNT = float(NTOT * HW)         # BN reduction count: 50176
EPS = 1e-5

F32 = mybir.dt.float32
F16 = mybir.dt.float16
F8 = mybir.dt.float8e4

ADD = mybir.AluOpType.add
MULT = mybir.AluOpType.mult

_cache = {}


USE_392 = False                # strided 392-col rhs vs padded 450-col rhs
SQ20_ON_ACT = True             # conv2-cb0 squares: ACT (True) or DVE (False)
CHUNK450 = 15 * PW             # 450 padded positions per 450-col chunk


def _chunk(nc, xs, wts, craw, cb, n, half, psum, scratch, sums, sumsqs,
           sq_on_act):
    """One binary-conv psum chunk (14 interior rows) + stats accumulation."""
    if USE_392:
        acc = psum.tile([P, COLS], F32, tag="acc")
        xv = xs[:, 2 * n:2 * n + 2, G:G + IMG].rearrange(
            "p a (r c) -> p a r c", c=PW)
        for k in range(9):
            kh, kw = divmod(k, 3)
            r0 = 14 * half + kh
            nc.tensor.matmul(
                acc,
                lhsT=wts[:, k, :, cb * P:(cb + 1) * P],
                rhs=xv[:, :, r0:r0 + 14, kw:kw + 28],
                start=(k == 0),
                stop=(k == 8),
                perf_mode=mybir.MatmulPerfMode.DoubleRow,
            )
        intr = acc
    else:
        acc = psum.tile([P, CHUNK450], F32, tag="acc")
        for k in range(9):
            kh, kw = divmod(k, 3)
            base = G + (15 * half + kh - 1) * PW + (kw - 1)
            nc.tensor.matmul(
                acc,
                lhsT=wts[:, k, :, cb * P:(cb + 1) * P],
                rhs=xs[:, 2 * n:2 * n + 2, base:base + CHUNK450],
                start=(k == 0),
                stop=(k == 8),
                perf_mode=mybir.MatmulPerfMode.DoubleRow,
            )
        rows = acc.rearrange("p (r c) -> p r c", c=PW)
        r_lo = 1 - half  # skip padded row 0 in the first chunk
        intr = rows[:, r_lo:r_lo + 14, 1:1 + W]
    ci = 2 * n + half
    sl = slice(half * COLS, (half + 1) * COLS)
    # copy to f16 staging + per-chunk channel sums (DVE)
    nc.vector.tensor_scalar(
        out=craw[:, cb, n, sl], in0=intr, scalar1=0.0, scalar2=0.0,
        op0=ADD, op1=ADD, accum_out=sums[:, ci:ci + 1],
    )
    # per-chunk channel sum-of-squares
    sq = scratch.tile([P, COLS], F32, tag="sq")
    if sq_on_act:
        nc.scalar.activation(
            sq, intr, mybir.ActivationFunctionType.Square,
            accum_out=sumsqs[:, ci:ci + 1],
        )
    else:
        # DVE square of the f16 staging copy (psum can't be read twice)
        nc.vector.scalar_tensor_tensor(
            sq, in0=craw[:, cb, n, sl], scalar=1.0, in1=craw[:, cb, n, sl],
            op0=MULT, op1=MULT, accum_out=sumsqs[:, ci:ci + 1],
        )


def _bn_coeffs(nc, small, s_col, q_col, g_t, b_t, eps_t, tag):
    """Global-stat BN coefficients: scale = g*rstd, bias = b - mean*scale."""
    mean = small.tile([P, 1], F32, name=f"mean{tag}", tag=f"mean{tag}")
    nc.vector.tensor_scalar_mul(mean, s_col, 1.0 / CNT)
    ex2 = small.tile([P, 1], F32, name=f"ex2{tag}", tag=f"ex2{tag}")
    nc.vector.tensor_scalar_mul(ex2, q_col, 1.0 / CNT)
    m2 = small.tile([P, 1], F32, name=f"m2{tag}", tag=f"m2{tag}")
    nc.vector.tensor_mul(m2, mean, mean)
    var = small.tile([P, 1], F32, name=f"var{tag}", tag=f"var{tag}")
    nc.vector.tensor_sub(var, ex2, m2)
    sd = small.tile([P, 1], F32, name=f"sd{tag}", tag=f"sd{tag}")
    nc.scalar.activation(sd, var, mybir.ActivationFunctionType.Sqrt,
                         bias=eps_t)
    rstd = small.tile([P, 1], F32, name=f"rstd{tag}", tag=f"rstd{tag}")
    nc.vector.reciprocal(rstd, sd)
    scale = small.tile([P, 1], F32, name=f"scale{tag}", tag=f"scale{tag}")
    nc.vector.tensor_mul(scale, g_t, rstd)
    ms = small.tile([P, 1], F32, name=f"ms{tag}", tag=f"ms{tag}")
    nc.vector.tensor_mul(ms, mean, scale)
    bias = small.tile([P, 1], F32, name=f"bias{tag}", tag=f"bias{tag}")
    nc.vector.tensor_sub(bias, b_t, ms)
    return scale, bias


def _memset_borders(nc, xs):
    """Zero the guard bands and the 1-px padding border of every plane."""
    nc.vector.memset(xs[:, :, 0:G], 0.0)                         # low guards
    nc.vector.memset(xs[:, :, G + IMG:], 0.0)                    # high guards
    nc.vector.memset(xs[:, :, G:G + PW], 0.0)                    # top rows
    nc.vector.memset(xs[:, :, G + IMG - PW:G + IMG], 0.0)        # bottom rows
    mid = xs[:, :, G + PW:G + IMG - PW].rearrange(
        "p a (r c) -> p a r c", c=PW)
    nc.vector.memset(mid[:, :, :, 0:1], 0.0)                     # left cols
    nc.vector.memset(mid[:, :, :, PW - 1:PW], 0.0)               # right cols


def _build():
    nc = bacc.Bacc("TRN2", target_bir_lowering=False, debug=False,
                   num_devices=N_CORES)

    x_d = nc.dram_tensor("x", [NPC, C, H, W], F32, kind="ExternalInput").ap()
    w1_d = nc.dram_tensor("w1p", [P, 9, J, C], F8, kind="ExternalInput").ap()
    w2_d = nc.dram_tensor("w2p", [P, 9, J, C], F8, kind="ExternalInput").ap()
    gb1_d = nc.dram_tensor("gb1", [2, J, P], F32, kind="ExternalInput").ap()
    gb2_d = nc.dram_tensor("gb2", [2, J, P], F32, kind="ExternalInput").ap()
    y_d = nc.dram_tensor("y", [NPC, C, H, W], F32, kind="ExternalOutput").ap()

    groups = [list(range(N_CORES))]

    with tile.TileContext(nc) as tc, ExitStack() as ctx:
        big = ctx.enter_context(tc.tile_pool(name="big", bufs=1))
        small = ctx.enter_context(tc.tile_pool(name="small", bufs=1))
        psum = ctx.enter_context(tc.tile_pool(name="psum", bufs=8,
                                              space="PSUM"))
        scratch = ctx.enter_context(tc.tile_pool(name="scratch", bufs=2))
        outp = ctx.enter_context(tc.tile_pool(name="outp", bufs=6))
        dram = ctx.enter_context(tc.tile_pool(name="dram", bufs=1,
                                              space="DRAM"))

        def _ar(st, tag):
            """ncfw AllReduce of a [P, k] stats tile; returns gathered tile.

            All three steps stay on the gpsimd queue: a cross-queue wait on a
            collective's completion has no reliable hardware semaphore path
            (hangs on HW), so in-queue ordering is load-bearing here.
            """
            k = st.shape[-1]
            ar_in = dram.tile([P, k], F32, name=f"ari{tag}")
            ar_out = dram.tile([P, k], F32, name=f"aro{tag}")
            nc.gpsimd.dma_start(out=ar_in, in_=st)
            nc.gpsimd.collective_compute(
                "AllReduce", ADD, replica_groups=groups,
                ins=[ar_in.opt()], outs=[ar_out.opt()],
            )
            stg = small.tile([P, k], F32, name=f"arg{tag}", tag=f"arg{tag}")
            nc.gpsimd.dma_start(out=stg, in_=ar_out)
            return stg

        def _emit_out(cb, ci, scale, bias, style):
            """BN2 + shortcut + store for one 392-col chunk."""
            n, half = divmod(ci, 2)
            sl = slice(half * COLS, (half + 1) * COLS)
            yt = outp.tile([P, COLS], F32, tag="yt")
            if style == "dve":  # scale/bias leg on DVE
                nc.vector.tensor_scalar(
                    out=yt, in0=c2raw[:, cb, n, sl],
                    scalar1=scale, scalar2=bias, op0=MULT, op1=ADD,
                )
            else:  # scale/bias leg on the ACT engine
                nc.scalar.activation(
                    yt, c2raw[:, cb, n, sl],
                    mybir.ActivationFunctionType.Identity,
                    bias=bias, scale=scale,
                )
            yo = outp.tile([P, COLS], F32, tag="yo")
            nc.vector.tensor_add(yo, yt, xstage[:, cb, n, sl])
            nc.sync.dma_start(
                out=y_d[n, cb * P:(cb + 1) * P].rearrange(
                    "p h w -> p (h w)")[:, sl],
                in_=yo,
            )

        # ---- dummy AllReduce first: absorbs the one-time ncfw collective
        # setup (~70us) under the input-DMA/conv1 window and gives the 8
        # cores a coordinated start.
        zs = small.tile([P, 1], F32, tag="zs")
        nc.vector.memset(zs, 0.0)
        dummy_in = dram.tile([P, 1], F32)
        dummy_out = dram.tile([P, 1], F32)
        nc.sync.dma_start(out=dummy_in, in_=zs)
        nc.gpsimd.collective_compute(
            "AllReduce", ADD, replica_groups=groups,
            ins=[dummy_in.opt()], outs=[dummy_out.opt()],
        )

        # ---- padded fp8 sign planes (borders zeroed once)
        xstage = big.tile([P, J, NPC, HW], F32)
        xs1 = big.tile([P, NPC * J, PLANE], F8)
        xs2 = big.tile([P, NPC * J, PLANE], F8)
        _memset_borders(nc, xs1)
        _memset_borders(nc, xs2)
        eps_t = small.tile([P, 1], F32, tag="eps")
        nc.vector.memset(eps_t, EPS)

        # ---- x in (image-major), sign to fp8; conv1 weights right after
        # image 0 so the first matmul can start ASAP
        w1s = big.tile([P, 9, J, C], F8)
        for n in range(NPC):
            for j in range(J):
                nc.sync.dma_start(
                    out=xstage[:, j, n, :],
                    in_=x_d[n, j * P:(j + 1) * P].rearrange(
                        "p h w -> p (h w)"),
                )
                interior = xs1[:, 2 * n + j, G:G + IMG].rearrange(
                    "p (r c) -> p r c", c=PW)[:, 1:1 + H, 1:1 + W]
                nc.scalar.sign(
                    interior,
                    xstage[:, j, n, :].rearrange("p (r c) -> p r c", c=W),
                )
            if n == 0:
                nc.sync.dma_start(out=w1s, in_=w1_d)

        w2s = big.tile([P, 9, J, C], F8)
        nc.sync.dma_start(out=w2s, in_=w2_d)
        gb_t = []
        for li, gb_d in enumerate((gb1_d, gb2_d)):
            per = []
            for j in range(J):
                g_t = small.tile([P, 1], F32, name=f"g{li}{j}",
                                 tag=f"g{li}{j}")
                b_t = small.tile([P, 1], F32, name=f"b{li}{j}",
                                 tag=f"b{li}{j}")
                nc.sync.dma_start(out=g_t,
                                  in_=gb_d[0, j].rearrange("(p o) -> p o",
                                                           o=1))
                nc.sync.dma_start(out=b_t,
                                  in_=gb_d[1, j].rearrange("(p o) -> p o",
                                                           o=1))
                per.append((g_t, b_t))
            gb_t.append(per)

        # ---- layer 1: conv block 0, AR under block 1's shadow, conv block 1
        c1raw = big.tile([P, J, NPC, HW], F16)
        c2raw = big.tile([P, J, NPC, HW], F16)
        sums10 = small.tile([P, 16], F32, tag="s10")
        sumsqs10 = small.tile([P, 16], F32, tag="q10")
        for n in range(NPC):
            for half in range(2):
                _chunk(nc, xs1, w1s, c1raw, 0, n, half, psum, scratch,
                       sums10, sumsqs10, sq_on_act=True)
        st10 = small.tile([P, 2], F32, tag="st10")
        nc.vector.reduce_sum(st10[:, 0:1], sums10, axis=mybir.AxisListType.X)
        nc.vector.reduce_sum(st10[:, 1:2], sumsqs10,
                             axis=mybir.AxisListType.X)
        stg10 = _ar(st10, "10")

        sums11 = small.tile([P, 16], F32, tag="s11")
        sumsqs11 = small.tile([P, 16], F32, tag="q11")
        sc1 = [None, None]
        for n in range(NPC):
            for half in range(2):
                _chunk(nc, xs1, w1s, c1raw, 1, n, half, psum, scratch,
                       sums11, sumsqs11, sq_on_act=True)
                if 2 * n + half == 5:
                    # block-0 BN1 coeffs in conv1-block1's shadow
                    sc1[0] = _bn_coeffs(nc, small, stg10[:, 0:1],
                                        stg10[:, 1:2], gb_t[0][0][0],
                                        gb_t[0][0][1], eps_t, "10")
        st11 = small.tile([P, 2], F32, tag="st11")
        nc.vector.reduce_sum(st11[:, 0:1], sums11, axis=mybir.AxisListType.X)
        nc.vector.reduce_sum(st11[:, 1:2], sumsqs11,
                             axis=mybir.AxisListType.X)
        stg11 = _ar(st11, "11")
        sc1[1] = _bn_coeffs(nc, small, stg11[:, 0:1], stg11[:, 1:2],
                            gb_t[0][1][0], gb_t[0][1][1], eps_t, "11")

        # ---- conv2 block 0, interlayer sign interleaved image-by-image
        sums20 = small.tile([P, 16], F32, tag="s20")
        sumsqs20 = small.tile([P, 16], F32, tag="q20")
        for n in range(NPC):
            for j in range(2):
                interior = xs2[:, 2 * n + j, G:G + IMG].rearrange(
                    "p (r c) -> p r c", c=PW)[:, 1:1 + H, 1:1 + W]
                nc.scalar.activation(
                    interior,
                    c1raw[:, j, n, :].rearrange("p (r c) -> p r c", c=W),
                    mybir.ActivationFunctionType.Sign,
                    bias=sc1[j][1], scale=sc1[j][0],
                )
            for half in range(2):
                _chunk(nc, xs2, w2s, c2raw, 0, n, half, psum, scratch,
                       sums20, sumsqs20, sq_on_act=SQ20_ON_ACT)
        st20 = small.tile([P, 2], F32, tag="st20")
        nc.vector.reduce_sum(st20[:, 0:1], sums20, axis=mybir.AxisListType.X)
        nc.vector.reduce_sum(st20[:, 1:2], sumsqs20,
                             axis=mybir.AxisListType.X)
        stg20 = _ar(st20, "20")

        # ---- conv2 block 1, with block-0 coeffs+outputs in its shadow
        sums21 = small.tile([P, 16], F32, tag="s21")
        sumsqs21 = small.tile([P, 16], F32, tag="q21")
        sc20 = None
        for n in range(NPC):
            for half in range(2):
                _chunk(nc, xs2, w2s, c2raw, 1, n, half, psum, scratch,
                       sums21, sumsqs21, sq_on_act=True)
                ci = 2 * n + half
                if ci == 5:
                    sc20 = _bn_coeffs(nc, small, stg20[:, 0:1], stg20[:, 1:2],
                                      gb_t[1][0][0], gb_t[1][0][1], eps_t,
                                      "20")
                if ci >= 7:
                    _emit_out(0, ci - 7, sc20[0], sc20[1], "act")
        st21 = small.tile([P, 2], F32, tag="st21")
        nc.vector.reduce_sum(st21[:, 0:1], sums21, axis=mybir.AxisListType.X)
        nc.vector.reduce_sum(st21[:, 1:2], sumsqs21,
                             axis=mybir.AxisListType.X)
        stg21 = _ar(st21, "21")

        # leftover block-0 outputs run during the AR mesh
        for ci in range(9, 16):
            _emit_out(0, ci, sc20[0], sc20[1], "act")
        sc21 = _bn_coeffs(nc, small, stg21[:, 0:1], stg21[:, 1:2],
                          gb_t[1][1][0], gb_t[1][1][1], eps_t, "21")
        for ci in range(16):
            _emit_out(1, ci, sc21[0], sc21[1], "dve" if ci < 2 else "act")

    nc.compile()
    return nc


def _pack_w(w):
    # [co, ci, kh, kw] -> sign -> [ci%128, kh*3+kw, ci//128, co] fp8e4
    s = np.sign(w.astype(np.float32)).reshape(C, J, P, 9)
    return np.ascontiguousarray(s.transpose(2, 3, 1, 0)).astype(
        ml_dtypes.float8_e4m3)


def _pack_gb(g, b):
    return np.ascontiguousarray(
        np.stack([g, b]).astype(np.float32).reshape(2, J, P))


def kernel(x, w1, g1, b1, w2, g2, b2, _profile=False):
    if "nc" not in _cache:
        _cache["nc"] = _build()
    nc = _cache["nc"]

    x = np.ascontiguousarray(x, np.float32)
    w1p, w2p = _pack_w(w1), _pack_w(w2)
    gb1, gb2 = _pack_gb(g1, b1), _pack_gb(g2, b2)
    in_maps = [
        {"x": x[c * NPC:(c + 1) * NPC], "w1p": w1p, "w2p": w2p,
         "gb1": gb1, "gb2": gb2}
        for c in range(N_CORES)
    ]
    res = bass_utils.run_bass_kernel_spmd(
        nc, in_maps, core_ids=list(range(N_CORES)), trace=_profile)
    y = np.concatenate([res.results[c]["y"] for c in range(N_CORES)], axis=0)
    if _profile:
        kernel.last_exec_time_ns = res.exec_time_ns
        kernel.last_results = res
    return y


# revision 15
# speedup vs baseline: 1.1800x; 1.0722x over previous
"""Trainium2 Bass kernel for a binarized (1w1a) ResNet BasicBlock.

  out = BN2(bconv3x3(sign(BN1(bconv3x3(sign(x), sign(w1))), g1, b1), sign(w2)), g2, b2) + x

with training-mode (sync) BatchNorm over (N, H, W) and identity shortcut.
Shapes: x [64, 256, 28, 28] f32, w [256, 256, 3, 3] f32, g/b [256] f32.

Strategy (8 NeuronCores, data-parallel over batch, 8 images/core):
  - conv3x3 = 9 shifted matmuls over a zero-padded 30x30 spatial layout.
    Activations are sign() in fp8e4 (+-1 exact); contraction over 256 input
    channels runs as a single fp8 DoubleRow matmul (K=128 partitions x 2).
    Each psum chunk computes only the 14x28 interior rows (392 cols) via a
    strided rhs AP - no wasted border columns.
  - BatchNorm needs global (sync) stats: per-chunk channel sum/sumsq are
    accumulated on the fly (DVE copy w/ accum_out + Square w/ accum_out),
    then all-reduced across the 8 cores via ncfw. Layer 1 uses ONE combined
    AllReduce for both channel blocks; layer 2 keeps per-block AllReduces so
    block 0's BN+shortcut+store overlaps block 1's conv. A dummy AllReduce
    issued at kernel start absorbs the expensive first-collective setup
    (~70us) under the conv1 window.
  - Weights are sign()ed and laid out host-side (negligible: 0.05% of FLOPs).
  - Scheduling: interlayer sign() is interleaved image-by-image with conv2
    so conv2 starts right after the first image's planes are binarized;
    block-0 BN2 coefficients and outputs are issued interleaved into the
    conv2-block-1 chunk loop so they run in its shadow; output DMA streams
    per chunk.
"""

import os
import sys

sys.path.insert(0, "/opt/trn_rl_repo")

import numpy as np
import ml_dtypes
from contextlib import ExitStack

import concourse.bass as bass
import concourse.tile as tile
from concourse import bacc, mybir
from concourse import bass_utils

N_CORES = 8
NTOT, C, H, W = 64, 256, 28, 28
NPC = NTOT // N_CORES          # images per core
P, J = 128, 2                  # partition block, channel blocks
PW = 30                        # padded width/height
IMG = PW * PW                  # 900
G = 32                         # front pad (keeps plane starts staggered)
PLANE = 1060                   # padded plane stride in the fp8 layout
HW = H * W                     # 784
COLS = 392                     # one psum chunk: 14 interior rows x 28 cols
CNT = float(NTOT * HW)         # BN reduction count: 50176
EPS = 1e-5

F32 = mybir.dt.float32
F16 = mybir.dt.float16
F8 = mybir.dt.float8e4

ADD = mybir.AluOpType.add
MULT = mybir.AluOpType.mult

_cache = {}


USE_392 = False                # strided 392-col rhs vs padded 450-col rhs
SQ20_ON_ACT = True             # conv2-cb0 squares: ACT (True) or DVE (False)
CHUNK450 = 15 * PW             # 450 padded positions per 450-col chunk


def _chunk(nc, xs, wts, craw, cb, n, half, psum, scratch, sums, sumsqs,
           sq_on_act):
    """One binary-conv psum chunk (14 interior rows) + stats accumulation."""
    if USE_392:
        acc = psum.tile([P, COLS], F32, tag="acc")
        xv = xs[:, 2 * n:2 * n + 2, G:G + IMG].rearrange(
            "p a (r c) -> p a r c", c=PW)
        for k in range(9):
            kh, kw = divmod(k, 3)
            r0 = 14 * half + kh
            nc.tensor.matmul(
                acc,
                lhsT=wts[:, k, :, cb * P:(cb + 1) * P],
                rhs=xv[:, :, r0:r0 + 14, kw:kw + 28],
                start=(k == 0),
                stop=(k == 8),
                perf_mode=mybir.MatmulPerfMode.DoubleRow,
            )
        intr = acc
    else:
        acc = psum.tile([P, CHUNK450], F32, tag="acc")
        for k in range(9):
            kh, kw = divmod(k, 3)
            base = G + (15 * half + kh - 1) * PW + (kw - 1)
            nc.tensor.matmul(
                acc,
                lhsT=wts[:, k, :, cb * P:(cb + 1) * P],
                rhs=xs[:, 2 * n:2 * n + 2, base:base + CHUNK450],
                start=(k == 0),
                stop=(k == 8),
                perf_mode=mybir.MatmulPerfMode.DoubleRow,
            )
        rows = acc.rearrange("p (r c) -> p r c", c=PW)
        r_lo = 1 - half  # skip padded row 0 in the first chunk
        intr = rows[:, r_lo:r_lo + 14, 1:1 + W]
    ci = 2 * n + half
    sl = slice(half * COLS, (half + 1) * COLS)
    # copy to f16 staging + per-chunk channel sums (DVE)
    nc.vector.tensor_scalar(
        out=craw[:, cb, n, sl], in0=intr, scalar1=0.0, scalar2=0.0,
        op0=ADD, op1=ADD, accum_out=sums[:, ci:ci + 1],
    )
    # per-chunk channel sum-of-squares
    sq = scratch.tile([P, COLS], F32, tag="sq")
    if sq_on_act:
        nc.scalar.activation(
            sq, intr, mybir.ActivationFunctionType.Square,
            accum_out=sumsqs[:, ci:ci + 1],
        )
    else:
        # DVE square of the f16 staging copy (psum can't be read twice)
        nc.vector.scalar_tensor_tensor(
            sq, in0=craw[:, cb, n, sl], scalar=1.0, in1=craw[:, cb, n, sl],
            op0=MULT, op1=MULT, accum_out=sumsqs[:, ci:ci + 1],
        )


def _bn_coeffs(nc, small, s_col, q_col, g_t, b_t, eps_t, tag):
    """Global-stat BN coefficients: scale = g*rstd, bias = b - mean*scale."""
    mean = small.tile([P, 1], F32, name=f"mean{tag}", tag=f"mean{tag}")
    nc.vector.tensor_scalar_mul(mean, s_col, 1.0 / CNT)
    ex2 = small.tile([P, 1], F32, name=f"ex2{tag}", tag=f"ex2{tag}")
    nc.vector.tensor_scalar_mul(ex2, q_col, 1.0 / CNT)
    m2 = small.tile([P, 1], F32, name=f"m2{tag}", tag=f"m2{tag}")
    nc.vector.tensor_mul(m2, mean, mean)
    var = small.tile([P, 1], F32, name=f"var{tag}", tag=f"var{tag}")
    nc.vector.tensor_sub(var, ex2, m2)
    sd = small.tile([P, 1], F32, name=f"sd{tag}", tag=f"sd{tag}")
    nc.scalar.activation(sd, var, mybir.ActivationFunctionType.Sqrt,
                         bias=eps_t)
    rstd = small.tile([P, 1], F32, name=f"rstd{tag}", tag=f"rstd{tag}")
    nc.vector.reciprocal(rstd, sd)
    scale = small.tile([P, 1], F32, name=f"scale{tag}", tag=f"scale{tag}")
    nc.vector.tensor_mul(scale, g_t, rstd)
    ms = small.tile([P, 1], F32, name=f"ms{tag}", tag=f"ms{tag}")
    nc.vector.tensor_mul(ms, mean, scale)
    bias = small.tile([P, 1], F32, name=f"bias{tag}", tag=f"bias{tag}")
    nc.vector.tensor_sub(bias, b_t, ms)
    return scale, bias


def _memset_borders(nc, xs):
    """Zero the guard bands and the 1-px padding border of every plane."""
    nc.vector.memset(xs[:, :, 0:G], 0.0)                         # low guards
    nc.vector.memset(xs[:, :, G + IMG:], 0.0)                    # high guards
    nc.vector.memset(xs[:, :, G:G + PW], 0.0)                    # top rows
    nc.vector.memset(xs[:, :, G + IMG - PW:G + IMG], 0.0)        # bottom rows
    mid = xs[:, :, G + PW:G + IMG - PW].rearrange(
        "p a (r c) -> p a r c", c=PW)
    nc.vector.memset(mid[:, :, :, 0:1], 0.0)                     # left cols
    nc.vector.memset(mid[:, :, :, PW - 1:PW], 0.0)               # right cols


def _build():
    nc = bacc.Bacc("TRN2", target_bir_lowering=False, debug=False,
                   num_devices=N_CORES)

    x_d = nc.dram_tensor("x", [NPC, C, H, W], F32, kind="ExternalInput").ap()
    w1_d = nc.dram_tensor("w1p", [P, 9, J, C], F8, kind="ExternalInput").ap()
    w2_d = nc.dram_tensor("w2p", [P, 9, J, C], F8, kind="ExternalInput").ap()
    gb1_d = nc.dram_tensor("gb1", [2, J, P], F32, kind="ExternalInput").ap()
    gb2_d = nc.dram_tensor("gb2", [2, J, P], F32, kind="ExternalInput").ap()
    y_d = nc.dram_tensor("y", [NPC, C, H, W], F32, kind="ExternalOutput").ap()

    groups = [list(range(N_CORES))]

    with tile.TileContext(nc) as tc, ExitStack() as ctx:
        big = ctx.enter_context(tc.tile_pool(name="big", bufs=1))
        small = ctx.enter_context(tc.tile_pool(name="small", bufs=1))
        psum = ctx.enter_context(tc.tile_pool(name="psum", bufs=8,
                                              space="PSUM"))
        scratch = ctx.enter_context(tc.tile_pool(name="scratch", bufs=2))
        outp = ctx.enter_context(tc.tile_pool(name="outp", bufs=6))
        dram = ctx.enter_context(tc.tile_pool(name="dram", bufs=1,
                                              space="DRAM"))

        def _ar(st, tag):
            """ncfw AllReduce of a [P, k] stats tile; returns gathered tile.

            All three steps stay on the gpsimd queue: a cross-queue wait on a
            collective's completion has no reliable hardware semaphore path
            (hangs on HW), so in-queue ordering is load-bearing here.
            """
            k = st.shape[-1]
            ar_in = dram.tile([P, k], F32, name=f"ari{tag}")
            ar_out = dram.tile([P, k], F32, name=f"aro{tag}")
            nc.sync.dma_start(out=ar_in, in_=st)
            nc.gpsimd.collective_compute(
                "AllReduce", ADD, replica_groups=groups,
                ins=[ar_in.opt()], outs=[ar_out.opt()],
            )
            # gather on the sync queue: the gpsimd queue holds the collective
            # ~15us past mesh completion, which would push the BN coeffs into
            # the tail
            stg = small.tile([P, k], F32, name=f"arg{tag}", tag=f"arg{tag}")
            nc.sync.dma_start(out=stg, in_=ar_out)
            return stg

        def _emit_out(cb, ci, scale, bias, style):
            """BN2 + shortcut + store for one 392-col chunk."""
            n, half = divmod(ci, 2)
            sl = slice(half * COLS, (half + 1) * COLS)
            yt = outp.tile([P, COLS], F32, tag="yt")
            if style == "dve":  # scale/bias leg on DVE
                nc.vector.tensor_scalar(
                    out=yt, in0=c2raw[:, cb, n, sl],
                    scalar1=scale, scalar2=bias, op0=MULT, op1=ADD,
                )
            else:  # scale/bias leg on the ACT engine
                nc.scalar.activation(
                    yt, c2raw[:, cb, n, sl],
                    mybir.ActivationFunctionType.Identity,
                    bias=bias, scale=scale,
                )
            yo = outp.tile([P, COLS], F32, tag="yo")
            nc.vector.tensor_add(yo, yt, xstage[:, cb, n, sl])
            nc.sync.dma_start(
                out=y_d[n, cb * P:(cb + 1) * P].rearrange(
                    "p h w -> p (h w)")[:, sl],
                in_=yo,
            )

        # ---- dummy AllReduce first: absorbs the one-time ncfw collective
        # setup (~70us) under the input-DMA/conv1 window and gives the 8
        # cores a coordinated start.
        zs = small.tile([P, 1], F32, tag="zs")
        nc.vector.memset(zs, 0.0)
        dummy_in = dram.tile([P, 1], F32)
        dummy_out = dram.tile([P, 1], F32)
        nc.sync.dma_start(out=dummy_in, in_=zs)
        nc.gpsimd.collective_compute(
            "AllReduce", ADD, replica_groups=groups,
            ins=[dummy_in.opt()], outs=[dummy_out.opt()],
        )

        # ---- padded fp8 sign planes (borders zeroed once)
        xstage = big.tile([P, J, NPC, HW], F32)
        xs1 = big.tile([P, NPC * J, PLANE], F8)
        xs2 = big.tile([P, NPC * J, PLANE], F8)
        _memset_borders(nc, xs1)
        _memset_borders(nc, xs2)
        eps_t = small.tile([P, 1], F32, tag="eps")
        nc.vector.memset(eps_t, EPS)

        # ---- x in (image-major), sign to fp8; conv1 weights right after
        # image 0 so the first matmul can start ASAP
        w1s = big.tile([P, 9, J, C], F8)
        for n in range(NPC):
            for j in range(J):
                nc.sync.dma_start(
                    out=xstage[:, j, n, :],
                    in_=x_d[n, j * P:(j + 1) * P].rearrange(
                        "p h w -> p (h w)"),
                )
                interior = xs1[:, 2 * n + j, G:G + IMG].rearrange(
                    "p (r c) -> p r c", c=PW)[:, 1:1 + H, 1:1 + W]
                nc.scalar.sign(
                    interior,
                    xstage[:, j, n, :].rearrange("p (r c) -> p r c", c=W),
                )
            if n == 0:
                nc.sync.dma_start(out=w1s, in_=w1_d)

        w2s = big.tile([P, 9, J, C], F8)
        nc.sync.dma_start(out=w2s, in_=w2_d)
        gb_t = []
        for li, gb_d in enumerate((gb1_d, gb2_d)):
            per = []
            for j in range(J):
                g_t = small.tile([P, 1], F32, name=f"g{li}{j}",
                                 tag=f"g{li}{j}")
                b_t = small.tile([P, 1], F32, name=f"b{li}{j}",
                                 tag=f"b{li}{j}")
                nc.sync.dma_start(out=g_t,
                                  in_=gb_d[0, j].rearrange("(p o) -> p o",
                                                           o=1))
                nc.sync.dma_start(out=b_t,
                                  in_=gb_d[1, j].rearrange("(p o) -> p o",
                                                           o=1))
                per.append((g_t, b_t))
            gb_t.append(per)

        # ---- layer 1: conv block 0, AR under block 1's shadow, conv block 1
        c1raw = big.tile([P, J, NPC, HW], F16)
        c2raw = big.tile([P, J, NPC, HW], F16)
        sums10 = small.tile([P, 16], F32, tag="s10")
        sumsqs10 = small.tile([P, 16], F32, tag="q10")
        for n in range(NPC):
            for half in range(2):
                _chunk(nc, xs1, w1s, c1raw, 0, n, half, psum, scratch,
                       sums10, sumsqs10, sq_on_act=True)
        st10 = small.tile([P, 2], F32, tag="st10")
        nc.vector.reduce_sum(st10[:, 0:1], sums10, axis=mybir.AxisListType.X)
        nc.vector.reduce_sum(st10[:, 1:2], sumsqs10,
                             axis=mybir.AxisListType.X)
        stg10 = _ar(st10, "10")

        sums11 = small.tile([P, 16], F32, tag="s11")
        sumsqs11 = small.tile([P, 16], F32, tag="q11")
        sc1 = [None, None]
        for n in range(NPC):
            for half in range(2):
                _chunk(nc, xs1, w1s, c1raw, 1, n, half, psum, scratch,
                       sums11, sumsqs11, sq_on_act=True)
                if 2 * n + half == 5:
                    # block-0 BN1 coeffs in conv1-block1's shadow
                    sc1[0] = _bn_coeffs(nc, small, stg10[:, 0:1],
                                        stg10[:, 1:2], gb_t[0][0][0],
                                        gb_t[0][0][1], eps_t, "10")
        st11 = small.tile([P, 2], F32, tag="st11")
        nc.vector.reduce_sum(st11[:, 0:1], sums11, axis=mybir.AxisListType.X)
        nc.vector.reduce_sum(st11[:, 1:2], sumsqs11,
                             axis=mybir.AxisListType.X)
        stg11 = _ar(st11, "11")
        sc1[1] = _bn_coeffs(nc, small, stg11[:, 0:1], stg11[:, 1:2],
                            gb_t[0][1][0], gb_t[0][1][1], eps_t, "11")

        # ---- conv2 block 0, interlayer sign interleaved image-by-image
        sums20 = small.tile([P, 16], F32, tag="s20")
        sumsqs20 = small.tile([P, 16], F32, tag="q20")
        for n in range(NPC):
            for j in range(2):
                interior = xs2[:, 2 * n + j, G:G + IMG].rearrange(
                    "p (r c) -> p r c", c=PW)[:, 1:1 + H, 1:1 + W]
                nc.scalar.activation(
                    interior,
                    c1raw[:, j, n, :].rearrange("p (r c) -> p r c", c=W),
                    mybir.ActivationFunctionType.Sign,
                    bias=sc1[j][1], scale=sc1[j][0],
                )
            for half in range(2):
                _chunk(nc, xs2, w2s, c2raw, 0, n, half, psum, scratch,
                       sums20, sumsqs20, sq_on_act=SQ20_ON_ACT)
        st20 = small.tile([P, 2], F32, tag="st20")
        nc.vector.reduce_sum(st20[:, 0:1], sums20, axis=mybir.AxisListType.X)
        nc.vector.reduce_sum(st20[:, 1:2], sumsqs20,
                             axis=mybir.AxisListType.X)
        stg20 = _ar(st20, "20")

        # ---- conv2 block 1, with block-0 coeffs+outputs in its shadow
        sums21 = small.tile([P, 16], F32, tag="s21")
        sumsqs21 = small.tile([P, 16], F32, tag="q21")
        sc20 = None
        for n in range(NPC):
            for half in range(2):
                _chunk(nc, xs2, w2s, c2raw, 1, n, half, psum, scratch,
                       sums21, sumsqs21, sq_on_act=True)
                ci = 2 * n + half
                if ci == 5:
                    sc20 = _bn_coeffs(nc, small, stg20[:, 0:1], stg20[:, 1:2],
                                      gb_t[1][0][0], gb_t[1][0][1], eps_t,
                                      "20")
                if ci >= 7:
                    _emit_out(0, ci - 7, sc20[0], sc20[1], "act")
        st21 = small.tile([P, 2], F32, tag="st21")
        nc.vector.reduce_sum(st21[:, 0:1], sums21, axis=mybir.AxisListType.X)
        nc.vector.reduce_sum(st21[:, 1:2], sumsqs21,
                             axis=mybir.AxisListType.X)
        stg21 = _ar(st21, "21")

        # leftover block-0 outputs run during the AR mesh
        for ci in range(9, 16):
            _emit_out(0, ci, sc20[0], sc20[1], "act")
        sc21 = _bn_coeffs(nc, small, stg21[:, 0:1], stg21[:, 1:2],
                          gb_t[1][1][0], gb_t[1][1][1], eps_t, "21")
        for ci in range(16):
            _emit_out(1, ci, sc21[0], sc21[1], "dve" if ci < 2 else "act")

    nc.compile()
    return nc


def _pack_w(w):
    # [co, ci, kh, kw] -> sign -> [ci%128, kh*3+kw, ci//128, co] fp8e4
    s = np.sign(w.astype(np.float32)).reshape(C, J, P, 9)
    return np.ascontiguousarray(s.transpose(2, 3, 1, 0)).astype(
        ml_dtypes.float8_e4m3)


def _pack_gb(g, b):
    return np.ascontiguousarray(
        np.stack([g, b]).astype(np.float32).reshape(2, J, P))


def kernel(x, w1, g1, b1, w2, g2, b2, _profile=False):
    if "nc" not in _cache:
        _cache["nc"] = _build()
    nc = _cache["nc"]

    x = np.ascontiguousarray(x, np.float32)
    w1p, w2p = _pack_w(w1), _pack_w(w2)
    gb1, gb2 = _pack_gb(g1, b1), _pack_gb(g2, b2)
    in_maps = [
        {"x": x[c * NPC:(c + 1) * NPC], "w1p": w1p, "w2p": w2p,
         "gb1": gb1, "gb2": gb2}
        for c in range(N_CORES)
    ]
    res = bass_utils.run_bass_kernel_spmd(
        nc, in_maps, core_ids=list(range(N_CORES)), trace=_profile)
    y = np.concatenate([res.results[c]["y"] for c in range(N_CORES)], axis=0)
    if _profile:
        kernel.last_exec_time_ns = res.exec_time_ns
        kernel.last_results = res
    return y


# revision 16
# speedup vs baseline: 1.2608x; 1.0684x over previous
"""Trainium2 Bass kernel for a binarized (1w1a) ResNet BasicBlock.

  out = BN2(bconv3x3(sign(BN1(bconv3x3(sign(x), sign(w1))), g1, b1), sign(w2)), g2, b2) + x

with training-mode (sync) BatchNorm over (N, H, W) and identity shortcut.
Shapes: x [64, 256, 28, 28] f32, w [256, 256, 3, 3] f32, g/b [256] f32.

Strategy (8 NeuronCores, data-parallel over batch, 8 images/core):
  - conv3x3 = 9 shifted matmuls over a zero-padded 30x30 spatial layout.
    Activations are sign() in fp8e4 (+-1 exact); contraction over 256 input
    channels runs as a single fp8 DoubleRow matmul (K=128 partitions x 2).
    Each psum chunk computes only the 14x28 interior rows (392 cols) via a
    strided rhs AP - no wasted border columns.
  - BatchNorm needs global (sync) stats: per-chunk channel sum/sumsq are
    accumulated on the fly (DVE copy w/ accum_out + Square w/ accum_out),
    then all-reduced across the 8 cores via ncfw. Layer 1 uses ONE combined
    AllReduce for both channel blocks; layer 2 keeps per-block AllReduces so
    block 0's BN+shortcut+store overlaps block 1's conv. A dummy AllReduce
    issued at kernel start absorbs the expensive first-collective setup
    (~70us) under the conv1 window.
  - Weights are sign()ed and laid out host-side (negligible: 0.05% of FLOPs).
  - Scheduling: interlayer sign() is interleaved image-by-image with conv2
    so conv2 starts right after the first image's planes are binarized;
    block-0 BN2 coefficients and outputs are issued interleaved into the
    conv2-block-1 chunk loop so they run in its shadow; output DMA streams
    per chunk.
"""

import os
import sys

sys.path.insert(0, "/opt/trn_rl_repo")

import numpy as np
import ml_dtypes
from contextlib import ExitStack

import concourse.bass as bass
import concourse.tile as tile
from concourse import bacc, mybir
from concourse import bass_utils

N_CORES = 8
NTOT, C, H, W = 64, 256, 28, 28
NPC = NTOT // N_CORES          # images per core
P, J = 128, 2                  # partition block, channel blocks
PW = 30                        # padded width/height
IMG = PW * PW                  # 900
G = 32                         # front pad (keeps plane starts staggered)
PLANE = 1060                   # padded plane stride in the fp8 layout
HW = H * W                     # 784
COLS = 392                     # one psum chunk: 14 interior rows x 28 cols
CNT = float(NTOT * HW)         # BN reduction count: 50176
EPS = 1e-5

F32 = mybir.dt.float32
F16 = mybir.dt.float16
F8 = mybir.dt.float8e4

ADD = mybir.AluOpType.add
MULT = mybir.AluOpType.mult

_cache = {}


USE_392 = True                # strided 392-col rhs vs padded 450-col rhs
SQ20_ON_ACT = True             # conv2-cb0 squares: ACT (True) or DVE (False)
CHUNK450 = 15 * PW             # 450 padded positions per 450-col chunk


def _chunk(nc, xs, wts, craw, cb, n, half, psum, scratch, sums, sumsqs,
           sq_on_act):
    """One binary-conv psum chunk (14 interior rows) + stats accumulation."""
    if USE_392:
        acc = psum.tile([P, COLS], F32, tag="acc")
        xv = xs[:, 2 * n:2 * n + 2, G:G + IMG].rearrange(
            "p a (r c) -> p a r c", c=PW)
        for k in range(9):
            kh, kw = divmod(k, 3)
            r0 = 14 * half + kh
            nc.tensor.matmul(
                acc,
                lhsT=wts[:, k, :, cb * P:(cb + 1) * P],
                rhs=xv[:, :, r0:r0 + 14, kw:kw + 28],
                start=(k == 0),
                stop=(k == 8),
                perf_mode=mybir.MatmulPerfMode.DoubleRow,
            )
        intr = acc
    else:
        acc = psum.tile([P, CHUNK450], F32, tag="acc")
        for k in range(9):
            kh, kw = divmod(k, 3)
            base = G + (15 * half + kh - 1) * PW + (kw - 1)
            nc.tensor.matmul(
                acc,
                lhsT=wts[:, k, :, cb * P:(cb + 1) * P],
                rhs=xs[:, 2 * n:2 * n + 2, base:base + CHUNK450],
                start=(k == 0),
                stop=(k == 8),
                perf_mode=mybir.MatmulPerfMode.DoubleRow,
            )
        rows = acc.rearrange("p (r c) -> p r c", c=PW)
        r_lo = 1 - half  # skip padded row 0 in the first chunk
        intr = rows[:, r_lo:r_lo + 14, 1:1 + W]
    ci = 2 * n + half
    sl = slice(half * COLS, (half + 1) * COLS)
    # copy to f16 staging + per-chunk channel sums (DVE)
    nc.vector.tensor_scalar(
        out=craw[:, cb, n, sl], in0=intr, scalar1=0.0, scalar2=0.0,
        op0=ADD, op1=ADD, accum_out=sums[:, ci:ci + 1],
    )
    # per-chunk channel sum-of-squares
    sq = scratch.tile([P, COLS], F32, tag="sq")
    if sq_on_act:
        nc.scalar.activation(
            sq, intr, mybir.ActivationFunctionType.Square,
            accum_out=sumsqs[:, ci:ci + 1],
        )
    else:
        # DVE square of the f16 staging copy (psum can't be read twice)
        nc.vector.scalar_tensor_tensor(
            sq, in0=craw[:, cb, n, sl], scalar=1.0, in1=craw[:, cb, n, sl],
            op0=MULT, op1=MULT, accum_out=sumsqs[:, ci:ci + 1],
        )


def _bn_coeffs(nc, small, s_col, q_col, g_t, b_t, eps_t, tag):
    """Global-stat BN coefficients: scale = g*rstd, bias = b - mean*scale."""
    mean = small.tile([P, 1], F32, name=f"mean{tag}", tag=f"mean{tag}")
    nc.vector.tensor_scalar_mul(mean, s_col, 1.0 / CNT)
    ex2 = small.tile([P, 1], F32, name=f"ex2{tag}", tag=f"ex2{tag}")
    nc.vector.tensor_scalar_mul(ex2, q_col, 1.0 / CNT)
    m2 = small.tile([P, 1], F32, name=f"m2{tag}", tag=f"m2{tag}")
    nc.vector.tensor_mul(m2, mean, mean)
    var = small.tile([P, 1], F32, name=f"var{tag}", tag=f"var{tag}")
    nc.vector.tensor_sub(var, ex2, m2)
    sd = small.tile([P, 1], F32, name=f"sd{tag}", tag=f"sd{tag}")
    nc.scalar.activation(sd, var, mybir.ActivationFunctionType.Sqrt,
                         bias=eps_t)
    rstd = small.tile([P, 1], F32, name=f"rstd{tag}", tag=f"rstd{tag}")
    nc.vector.reciprocal(rstd, sd)
    scale = small.tile([P, 1], F32, name=f"scale{tag}", tag=f"scale{tag}")
    nc.vector.tensor_mul(scale, g_t, rstd)
    ms = small.tile([P, 1], F32, name=f"ms{tag}", tag=f"ms{tag}")
    nc.vector.tensor_mul(ms, mean, scale)
    bias = small.tile([P, 1], F32, name=f"bias{tag}", tag=f"bias{tag}")
    nc.vector.tensor_sub(bias, b_t, ms)
    return scale, bias


def _memset_borders(nc, xs):
    """Zero the guard bands and the 1-px padding border of every plane."""
    nc.vector.memset(xs[:, :, 0:G], 0.0)                         # low guards
    nc.vector.memset(xs[:, :, G + IMG:], 0.0)                    # high guards
    nc.vector.memset(xs[:, :, G:G + PW], 0.0)                    # top rows
    nc.vector.memset(xs[:, :, G + IMG - PW:G + IMG], 0.0)        # bottom rows
    mid = xs[:, :, G + PW:G + IMG - PW].rearrange(
        "p a (r c) -> p a r c", c=PW)
    nc.vector.memset(mid[:, :, :, 0:1], 0.0)                     # left cols
    nc.vector.memset(mid[:, :, :, PW - 1:PW], 0.0)               # right cols


def _build():
    nc = bacc.Bacc("TRN2", target_bir_lowering=False, debug=False,
                   num_devices=N_CORES)

    x_d = nc.dram_tensor("x", [NPC, C, H, W], F32, kind="ExternalInput").ap()
    w1_d = nc.dram_tensor("w1p", [P, 9, J, C], F8, kind="ExternalInput").ap()
    w2_d = nc.dram_tensor("w2p", [P, 9, J, C], F8, kind="ExternalInput").ap()
    gb1_d = nc.dram_tensor("gb1", [2, J, P], F32, kind="ExternalInput").ap()
    gb2_d = nc.dram_tensor("gb2", [2, J, P], F32, kind="ExternalInput").ap()
    y_d = nc.dram_tensor("y", [NPC, C, H, W], F32, kind="ExternalOutput").ap()

    groups = [list(range(N_CORES))]

    with tile.TileContext(nc) as tc, ExitStack() as ctx:
        big = ctx.enter_context(tc.tile_pool(name="big", bufs=1))
        small = ctx.enter_context(tc.tile_pool(name="small", bufs=1))
        psum = ctx.enter_context(tc.tile_pool(name="psum", bufs=8,
                                              space="PSUM"))
        scratch = ctx.enter_context(tc.tile_pool(name="scratch", bufs=2))
        outp = ctx.enter_context(tc.tile_pool(name="outp", bufs=6))
        dram = ctx.enter_context(tc.tile_pool(name="dram", bufs=1,
                                              space="DRAM"))

        def _ar(st, tag):
            """ncfw AllReduce of a [P, k] stats tile; returns gathered tile.

            All three steps stay on the gpsimd queue: a cross-queue wait on a
            collective's completion has no reliable hardware semaphore path
            (hangs on HW), so in-queue ordering is load-bearing here.
            """
            k = st.shape[-1]
            ar_in = dram.tile([P, k], F32, name=f"ari{tag}")
            ar_out = dram.tile([P, k], F32, name=f"aro{tag}")
            nc.sync.dma_start(out=ar_in, in_=st)
            nc.gpsimd.collective_compute(
                "AllReduce", ADD, replica_groups=groups,
                ins=[ar_in.opt()], outs=[ar_out.opt()],
            )
            # gather on the sync queue: the gpsimd queue holds the collective
            # ~15us past mesh completion, which would push the BN coeffs into
            # the tail
            stg = small.tile([P, k], F32, name=f"arg{tag}", tag=f"arg{tag}")
            nc.sync.dma_start(out=stg, in_=ar_out)
            return stg

        def _emit_out(cb, ci, scale, bias, style):
            """BN2 + shortcut + store for one 392-col chunk."""
            n, half = divmod(ci, 2)
            sl = slice(half * COLS, (half + 1) * COLS)
            yt = outp.tile([P, COLS], F32, tag="yt")
            if style == "dve":  # scale/bias leg on DVE
                nc.vector.tensor_scalar(
                    out=yt, in0=c2raw[:, cb, n, sl],
                    scalar1=scale, scalar2=bias, op0=MULT, op1=ADD,
                )
            else:  # scale/bias leg on the ACT engine
                nc.scalar.activation(
                    yt, c2raw[:, cb, n, sl],
                    mybir.ActivationFunctionType.Identity,
                    bias=bias, scale=scale,
                )
            yo = outp.tile([P, COLS], F32, tag="yo")
            nc.vector.tensor_add(yo, yt, xstage[:, cb, n, sl])
            nc.sync.dma_start(
                out=y_d[n, cb * P:(cb + 1) * P].rearrange(
                    "p h w -> p (h w)")[:, sl],
                in_=yo,
            )

        # ---- dummy AllReduce first: absorbs the one-time ncfw collective
        # setup (~70us) under the input-DMA/conv1 window and gives the 8
        # cores a coordinated start.
        zs = small.tile([P, 1], F32, tag="zs")
        nc.vector.memset(zs, 0.0)
        dummy_in = dram.tile([P, 1], F32)
        dummy_out = dram.tile([P, 1], F32)
        nc.sync.dma_start(out=dummy_in, in_=zs)
        nc.gpsimd.collective_compute(
            "AllReduce", ADD, replica_groups=groups,
            ins=[dummy_in.opt()], outs=[dummy_out.opt()],
        )

        # ---- padded fp8 sign planes (borders zeroed once)
        xstage = big.tile([P, J, NPC, HW], F32)
        xs1 = big.tile([P, NPC * J, PLANE], F8)
        xs2 = big.tile([P, NPC * J, PLANE], F8)
        _memset_borders(nc, xs1)
        _memset_borders(nc, xs2)
        eps_t = small.tile([P, 1], F32, tag="eps")
        nc.vector.memset(eps_t, EPS)

        # ---- x in (image-major), sign to fp8; conv1 weights right after
        # image 0 so the first matmul can start ASAP
        w1s = big.tile([P, 9, J, C], F8)
        for n in range(NPC):
            for j in range(J):
                nc.sync.dma_start(
                    out=xstage[:, j, n, :],
                    in_=x_d[n, j * P:(j + 1) * P].rearrange(
                        "p h w -> p (h w)"),
                )
                interior = xs1[:, 2 * n + j, G:G + IMG].rearrange(
                    "p (r c) -> p r c", c=PW)[:, 1:1 + H, 1:1 + W]
                nc.scalar.sign(
                    interior,
                    xstage[:, j, n, :].rearrange("p (r c) -> p r c", c=W),
                )
            if n == 0:
                nc.sync.dma_start(out=w1s, in_=w1_d)

        w2s = big.tile([P, 9, J, C], F8)
        nc.sync.dma_start(out=w2s, in_=w2_d)
        gb_t = []
        for li, gb_d in enumerate((gb1_d, gb2_d)):
            per = []
            for j in range(J):
                g_t = small.tile([P, 1], F32, name=f"g{li}{j}",
                                 tag=f"g{li}{j}")
                b_t = small.tile([P, 1], F32, name=f"b{li}{j}",
                                 tag=f"b{li}{j}")
                nc.sync.dma_start(out=g_t,
                                  in_=gb_d[0, j].rearrange("(p o) -> p o",
                                                           o=1))
                nc.sync.dma_start(out=b_t,
                                  in_=gb_d[1, j].rearrange("(p o) -> p o",
                                                           o=1))
                per.append((g_t, b_t))
            gb_t.append(per)

        # ---- layer 1: conv block 0, AR under block 1's shadow, conv block 1
        c1raw = big.tile([P, J, NPC, HW], F16)
        c2raw = big.tile([P, J, NPC, HW], F16)
        sums10 = small.tile([P, 16], F32, tag="s10")
        sumsqs10 = small.tile([P, 16], F32, tag="q10")
        for n in range(NPC):
            for half in range(2):
                _chunk(nc, xs1, w1s, c1raw, 0, n, half, psum, scratch,
                       sums10, sumsqs10, sq_on_act=True)
        st10 = small.tile([P, 2], F32, tag="st10")
        nc.vector.reduce_sum(st10[:, 0:1], sums10, axis=mybir.AxisListType.X)
        nc.vector.reduce_sum(st10[:, 1:2], sumsqs10,
                             axis=mybir.AxisListType.X)
        stg10 = _ar(st10, "10")

        sums11 = small.tile([P, 16], F32, tag="s11")
        sumsqs11 = small.tile([P, 16], F32, tag="q11")
        sc1 = [None, None]
        for n in range(NPC):
            for half in range(2):
                _chunk(nc, xs1, w1s, c1raw, 1, n, half, psum, scratch,
                       sums11, sumsqs11, sq_on_act=True)
                if 2 * n + half == 5:
                    # block-0 BN1 coeffs in conv1-block1's shadow
                    sc1[0] = _bn_coeffs(nc, small, stg10[:, 0:1],
                                        stg10[:, 1:2], gb_t[0][0][0],
                                        gb_t[0][0][1], eps_t, "10")
        st11 = small.tile([P, 2], F32, tag="st11")
        nc.vector.reduce_sum(st11[:, 0:1], sums11, axis=mybir.AxisListType.X)
        nc.vector.reduce_sum(st11[:, 1:2], sumsqs11,
                             axis=mybir.AxisListType.X)
        stg11 = _ar(st11, "11")
        sc1[1] = _bn_coeffs(nc, small, stg11[:, 0:1], stg11[:, 1:2],
                            gb_t[0][1][0], gb_t[0][1][1], eps_t, "11")

        # ---- conv2 block 0, interlayer sign interleaved image-by-image
        sums20 = small.tile([P, 16], F32, tag="s20")
        sumsqs20 = small.tile([P, 16], F32, tag="q20")
        for n in range(NPC):
            for j in range(2):
                interior = xs2[:, 2 * n + j, G:G + IMG].rearrange(
                    "p (r c) -> p r c", c=PW)[:, 1:1 + H, 1:1 + W]
                nc.scalar.activation(
                    interior,
                    c1raw[:, j, n, :].rearrange("p (r c) -> p r c", c=W),
                    mybir.ActivationFunctionType.Sign,
                    bias=sc1[j][1], scale=sc1[j][0],
                )
            for half in range(2):
                _chunk(nc, xs2, w2s, c2raw, 0, n, half, psum, scratch,
                       sums20, sumsqs20, sq_on_act=SQ20_ON_ACT)
        st20 = small.tile([P, 2], F32, tag="st20")
        nc.vector.reduce_sum(st20[:, 0:1], sums20, axis=mybir.AxisListType.X)
        nc.vector.reduce_sum(st20[:, 1:2], sumsqs20,
                             axis=mybir.AxisListType.X)
        stg20 = _ar(st20, "20")

        # ---- conv2 block 1, with block-0 coeffs+outputs in its shadow
        sums21 = small.tile([P, 16], F32, tag="s21")
        sumsqs21 = small.tile([P, 16], F32, tag="q21")
        sc20 = None
        for n in range(NPC):
            for half in range(2):
                _chunk(nc, xs2, w2s, c2raw, 1, n, half, psum, scratch,
                       sums21, sumsqs21, sq_on_act=True)
                ci = 2 * n + half
                if ci == 5:
                    sc20 = _bn_coeffs(nc, small, stg20[:, 0:1], stg20[:, 1:2],
                                      gb_t[1][0][0], gb_t[1][0][1], eps_t,
                                      "20")
                if ci >= 7:
                    _emit_out(0, ci - 7, sc20[0], sc20[1], "act")
        st21 = small.tile([P, 2], F32, tag="st21")
        nc.vector.reduce_sum(st21[:, 0:1], sums21, axis=mybir.AxisListType.X)
        nc.vector.reduce_sum(st21[:, 1:2], sumsqs21,
                             axis=mybir.AxisListType.X)
        stg21 = _ar(st21, "21")

        # leftover block-0 outputs run during the AR mesh
        for ci in range(9, 16):
            _emit_out(0, ci, sc20[0], sc20[1], "act")
        sc21 = _bn_coeffs(nc, small, stg21[:, 0:1], stg21[:, 1:2],
                          gb_t[1][1][0], gb_t[1][1][1], eps_t, "21")
        for ci in range(16):
            _emit_out(1, ci, sc21[0], sc21[1], "dve" if ci < 2 else "act")

    nc.compile()
    return nc


def _pack_w(w):
    # [co, ci, kh, kw] -> sign -> [ci%128, kh*3+kw, ci//128, co] fp8e4
    s = np.sign(w.astype(np.float32)).reshape(C, J, P, 9)
    return np.ascontiguousarray(s.transpose(2, 3, 1, 0)).astype(
        ml_dtypes.float8_e4m3)


def _pack_gb(g, b):
    return np.ascontiguousarray(
        np.stack([g, b]).astype(np.float32).reshape(2, J, P))


def kernel(x, w1, g1, b1, w2, g2, b2, _profile=False):
    if "nc" not in _cache:
        _cache["nc"] = _build()
    nc = _cache["nc"]

    x = np.ascontiguousarray(x, np.float32)
    w1p, w2p = _pack_w(w1), _pack_w(w2)
    gb1, gb2 = _pack_gb(g1, b1), _pack_gb(g2, b2)
    in_maps = [
        {"x": x[c * NPC:(c + 1) * NPC], "w1p": w1p, "w2p": w2p,
         "gb1": gb1, "gb2": gb2}
        for c in range(N_CORES)
    ]
    res = bass_utils.run_bass_kernel_spmd(
        nc, in_maps, core_ids=list(range(N_CORES)), trace=_profile)
    y = np.concatenate([res.results[c]["y"] for c in range(N_CORES)], axis=0)
    if _profile:
        kernel.last_exec_time_ns = res.exec_time_ns
        kernel.last_results = res
    return y


# revision 26
# speedup vs baseline: 1.2728x; 1.0096x over previous
"""Trainium2 Bass kernel for a binarized (1w1a) ResNet BasicBlock.

  out = BN2(bconv3x3(sign(BN1(bconv3x3(sign(x), sign(w1))), g1, b1), sign(w2)), g2, b2) + x

with training-mode (sync) BatchNorm over (N, H, W) and identity shortcut.
Shapes: x [64, 256, 28, 28] f32, w [256, 256, 3, 3] f32, g/b [256] f32.

Strategy (8 NeuronCores, data-parallel over batch, 8 images/core):
  - conv3x3 = 9 shifted matmuls over a zero-padded 30x30 spatial layout.
    Activations are sign() in fp8e4 (+-1 exact); contraction over 256 input
    channels runs as a single fp8 DoubleRow matmul (K=128 partitions x 2).
    Each psum chunk computes only the 14x28 interior rows (392 cols) via a
    strided rhs AP - no wasted border columns.
  - BatchNorm needs global (sync) stats: per-chunk channel sum/sumsq are
    accumulated on the fly (DVE copy w/ accum_out + Square w/ accum_out),
    then all-reduced across the 8 cores via ncfw. Layer 1 uses ONE combined
    AllReduce for both channel blocks; layer 2 keeps per-block AllReduces so
    block 0's BN+shortcut+store overlaps block 1's conv. A dummy AllReduce
    issued at kernel start absorbs the expensive first-collective setup
    (~70us) under the conv1 window.
  - Weights are sign()ed and laid out host-side (negligible: 0.05% of FLOPs).
  - Scheduling: interlayer sign() is interleaved image-by-image with conv2
    so conv2 starts right after the first image's planes are binarized;
    block-0 BN2 coefficients and outputs are issued interleaved into the
    conv2-block-1 chunk loop so they run in its shadow; output DMA streams
    per chunk.
"""

import os
import sys

sys.path.insert(0, "/opt/trn_rl_repo")

import numpy as np
import ml_dtypes
from contextlib import ExitStack

import concourse.bass as bass
import concourse.tile as tile
from concourse import bacc, mybir
from concourse import bass_utils

N_CORES = 8
NTOT, C, H, W = 64, 256, 28, 28
NPC = NTOT // N_CORES          # images per core
P, J = 128, 2                  # partition block, channel blocks
PW = 30                        # padded width/height
IMG = PW * PW                  # 900
G = 32                         # front pad (keeps plane starts staggered)
PLANE = 1060                   # padded plane stride in the fp8 layout
HW = H * W                     # 784
COLS = 392                     # one psum chunk: 14 interior rows x 28 cols
CNT = float(NTOT * HW)         # BN reduction count: 50176
EPS = 1e-5

F32 = mybir.dt.float32
F16 = mybir.dt.float16
F8 = mybir.dt.float8e4

ADD = mybir.AluOpType.add
MULT = mybir.AluOpType.mult

_cache = {}


USE_392 = True                 # strided 392-col rhs vs padded 450-col rhs
SQ20_ON_ACT = True             # conv2-cb0 squares: ACT (True) or DVE (False)
RDMA_AR = False                 # remote-DMA stats exchange vs ncfw AllReduce
CHUNK450 = 15 * PW             # 450 padded positions per 450-col chunk


def _chunk(nc, xs, wts, craw, cb, n, half, psum, scratch, sums, sumsqs,
           sq_on_act):
    """One binary-conv psum chunk (14 interior rows) + stats accumulation."""
    if USE_392:
        acc = psum.tile([P, COLS], F32, tag="acc")
        xv = xs[:, 2 * n:2 * n + 2, G:G + IMG].rearrange(
            "p a (r c) -> p a r c", c=PW)
        for k in range(9):
            kh, kw = divmod(k, 3)
            r0 = 14 * half + kh
            nc.tensor.matmul(
                acc,
                lhsT=wts[:, k, :, cb * P:(cb + 1) * P],
                rhs=xv[:, :, r0:r0 + 14, kw:kw + 28],
                start=(k == 0),
                stop=(k == 8),
                perf_mode=mybir.MatmulPerfMode.DoubleRow,
            )
        intr = acc
    else:
        acc = psum.tile([P, CHUNK450], F32, tag="acc")
        for k in range(9):
            kh, kw = divmod(k, 3)
            base = G + (15 * half + kh - 1) * PW + (kw - 1)
            nc.tensor.matmul(
                acc,
                lhsT=wts[:, k, :, cb * P:(cb + 1) * P],
                rhs=xs[:, 2 * n:2 * n + 2, base:base + CHUNK450],
                start=(k == 0),
                stop=(k == 8),
                perf_mode=mybir.MatmulPerfMode.DoubleRow,
            )
        rows = acc.rearrange("p (r c) -> p r c", c=PW)
        r_lo = 1 - half  # skip padded row 0 in the first chunk
        intr = rows[:, r_lo:r_lo + 14, 1:1 + W]
    ci = 2 * n + half
    sl = slice(half * COLS, (half + 1) * COLS)
    # copy to f16 staging + per-chunk channel sums (DVE)
    nc.vector.tensor_scalar(
        out=craw[:, cb, n, sl], in0=intr, scalar1=0.0, scalar2=0.0,
        op0=ADD, op1=ADD, accum_out=sums[:, ci:ci + 1],
    )
    # per-chunk channel sum-of-squares
    sq = scratch.tile([P, COLS], F32, tag="sq")
    if sq_on_act:
        nc.scalar.activation(
            sq, intr, mybir.ActivationFunctionType.Square,
            accum_out=sumsqs[:, ci:ci + 1],
        )
    else:
        # DVE square of the f16 staging copy (psum can't be read twice)
        nc.vector.scalar_tensor_tensor(
            sq, in0=craw[:, cb, n, sl], scalar=1.0, in1=craw[:, cb, n, sl],
            op0=MULT, op1=MULT, accum_out=sumsqs[:, ci:ci + 1],
        )


def _bn_coeffs(nc, small, s_col, q_col, g_t, b_t, eps_t, tag):
    """Global-stat BN coefficients: scale = g*rstd, bias = b - mean*scale."""
    mean = small.tile([P, 1], F32, name=f"mean{tag}", tag=f"mean{tag}")
    nc.vector.tensor_scalar_mul(mean, s_col, 1.0 / CNT)
    ex2 = small.tile([P, 1], F32, name=f"ex2{tag}", tag=f"ex2{tag}")
    nc.vector.tensor_scalar_mul(ex2, q_col, 1.0 / CNT)
    m2 = small.tile([P, 1], F32, name=f"m2{tag}", tag=f"m2{tag}")
    nc.vector.tensor_mul(m2, mean, mean)
    var = small.tile([P, 1], F32, name=f"var{tag}", tag=f"var{tag}")
    nc.vector.tensor_sub(var, ex2, m2)
    sd = small.tile([P, 1], F32, name=f"sd{tag}", tag=f"sd{tag}")
    nc.scalar.activation(sd, var, mybir.ActivationFunctionType.Sqrt,
                         bias=eps_t)
    rstd = small.tile([P, 1], F32, name=f"rstd{tag}", tag=f"rstd{tag}")
    nc.vector.reciprocal(rstd, sd)
    scale = small.tile([P, 1], F32, name=f"scale{tag}", tag=f"scale{tag}")
    nc.vector.tensor_mul(scale, g_t, rstd)
    ms = small.tile([P, 1], F32, name=f"ms{tag}", tag=f"ms{tag}")
    nc.vector.tensor_mul(ms, mean, scale)
    bias = small.tile([P, 1], F32, name=f"bias{tag}", tag=f"bias{tag}")
    nc.vector.tensor_sub(bias, b_t, ms)
    return scale, bias


def _memset_borders(nc, xs):
    """Zero the guard bands and the 1-px padding border of every plane."""
    nc.vector.memset(xs[:, :, 0:G], 0.0)                         # low guards
    nc.vector.memset(xs[:, :, G + IMG:], 0.0)                    # high guards
    nc.vector.memset(xs[:, :, G:G + PW], 0.0)                    # top rows
    nc.vector.memset(xs[:, :, G + IMG - PW:G + IMG], 0.0)        # bottom rows
    mid = xs[:, :, G + PW:G + IMG - PW].rearrange(
        "p a (r c) -> p a r c", c=PW)
    nc.vector.memset(mid[:, :, :, 0:1], 0.0)                     # left cols
    nc.vector.memset(mid[:, :, :, PW - 1:PW], 0.0)               # right cols


def _build():
    nc = bacc.Bacc("TRN2", target_bir_lowering=False, debug=False,
                   num_devices=N_CORES)

    x_d = nc.dram_tensor("x", [NPC, C, H, W], F32, kind="ExternalInput").ap()
    w1_d = nc.dram_tensor("w1p", [P, 9, J, C], F8, kind="ExternalInput").ap()
    w2_d = nc.dram_tensor("w2p", [P, 9, J, C], F8, kind="ExternalInput").ap()
    gb1_d = nc.dram_tensor("gb1", [2, J, P], F32, kind="ExternalInput").ap()
    gb2_d = nc.dram_tensor("gb2", [2, J, P], F32, kind="ExternalInput").ap()
    y_d = nc.dram_tensor("y", [NPC, C, H, W], F32, kind="ExternalOutput").ap()

    groups = [list(range(N_CORES))]

    with tile.TileContext(nc) as tc, ExitStack() as ctx:
        big = ctx.enter_context(tc.tile_pool(name="big", bufs=1))
        small = ctx.enter_context(tc.tile_pool(name="small", bufs=1))
        psum = ctx.enter_context(tc.tile_pool(name="psum", bufs=8,
                                              space="PSUM"))
        scratch = ctx.enter_context(tc.tile_pool(name="scratch", bufs=2))
        outp = ctx.enter_context(tc.tile_pool(name="outp", bufs=6))
        dram = ctx.enter_context(tc.tile_pool(name="dram", bufs=1,
                                              space="DRAM"))

        if RDMA_AR:
            rsems = {t: nc.alloc_semaphore(f"rs_{t}")
                     for t in ("10", "11", "20", "21")}
            lsem = nc.alloc_semaphore("ls_all")
            for s in list(rsems.values()) + [lsem]:
                nc.gpsimd.sem_clear(s)
        recvs = {}

        def _ar_send(st, tag):
            """Start the cross-core stats exchange for a [P, k] tile.

            Each core remote-DMA-broadcasts its stats to peer (me XOR d)'s
            slot d, for d = 1..7; slot 0 is a local copy. Non-blocking: the
            matching _ar_recv waits for the 7 remote arrivals (sem += 2 per
            sender) and reduces the 8 slots.
            """
            k = st.shape[-1]
            if not RDMA_AR:
                ar_in = dram.tile([P, k], F32, name=f"ari{tag}")
                ar_out = dram.tile([P, k], F32, name=f"aro{tag}")
                nc.sync.dma_start(out=ar_in, in_=st)
                nc.gpsimd.collective_compute(
                    "AllReduce", ADD, replica_groups=groups,
                    ins=[ar_in.opt()], outs=[ar_out.opt()],
                )
                recvs[tag] = ar_out
                return
            recv = small.tile([P, 8, k], F32, name=f"rv{tag}", tag=f"rv{tag}")
            nc.vector.tensor_scalar_add(recv[:, 0, :], st, 0.0)
            for d in range(1, 8):
                rdests = [None] * 8
                rdests[d] = (0, d)
                nc.gpsimd.remote_dma_broadcast(
                    recv[:, d, :], st, remote_sem=rsems[tag],
                    local_sem=lsem, rdests=rdests)
            nc.gpsimd.trigger_dma(count=None)
            recvs[tag] = recv

        def _ar_recv(tag, k):
            """Blocking half of the exchange; returns the [P, k] global sums."""
            stg = small.tile([P, k], F32, name=f"arg{tag}", tag=f"arg{tag}")
            if not RDMA_AR:
                nc.sync.dma_start(out=stg, in_=recvs[tag])
                return stg
            nc.vector.wait_ge(rsems[tag], 14)
            recv = recvs[tag]
            for j in range(k):
                nc.vector.reduce_sum(stg[:, j:j + 1], recv[:, :, j],
                                     axis=mybir.AxisListType.X)
            return stg

        def _emit_out(cb, ci, scale, bias, style):
            """BN2 + shortcut + store for one 392-col chunk."""
            n, half = divmod(ci, 2)
            sl = slice(half * COLS, (half + 1) * COLS)
            yt = outp.tile([P, COLS], F32, tag="yt")
            if style == "dve":  # scale/bias leg on DVE
                nc.vector.tensor_scalar(
                    out=yt, in0=c2raw[:, cb, n, sl],
                    scalar1=scale, scalar2=bias, op0=MULT, op1=ADD,
                )
            else:  # scale/bias leg on the ACT engine
                nc.scalar.activation(
                    yt, c2raw[:, cb, n, sl],
                    mybir.ActivationFunctionType.Identity,
                    bias=bias, scale=scale,
                )
            yo = outp.tile([P, COLS], F32, tag="yo")
            nc.vector.tensor_add(yo, yt, xstage[:, cb, n, sl])
            nc.sync.dma_start(
                out=y_d[n, cb * P:(cb + 1) * P].rearrange(
                    "p h w -> p (h w)")[:, sl],
                in_=yo,
            )

        if not RDMA_AR:
            # dummy AllReduce first: absorbs the one-time ncfw collective
            # setup (~70us) under the input-DMA/conv1 window and gives the 8
            # cores a coordinated start.
            zs = small.tile([P, 1], F32, tag="zs")
            nc.vector.memset(zs, 0.0)
            dummy_in = dram.tile([P, 1], F32)
            dummy_out = dram.tile([P, 1], F32)
            nc.sync.dma_start(out=dummy_in, in_=zs)
            nc.gpsimd.collective_compute(
                "AllReduce", ADD,
                replica_groups=[[2 * i, 2 * i + 1] for i in range(4)],
                ins=[dummy_in.opt()], outs=[dummy_out.opt()],
            )

        # ---- padded fp8 sign planes (borders zeroed once)
        xstage = big.tile([P, J, NPC, HW], F32)
        xs1 = big.tile([P, NPC * J, PLANE], F8)
        xs2 = big.tile([P, NPC * J, PLANE], F8)
        _memset_borders(nc, xs1)
        _memset_borders(nc, xs2)
        eps_t = small.tile([P, 1], F32, tag="eps")
        nc.vector.memset(eps_t, EPS)

        # ---- x in (image-major), sign to fp8; conv1 weights right after
        # image 0 so the first matmul can start ASAP
        w1s = big.tile([P, 9, J, C], F8)
        for n in range(NPC):
            for j in range(J):
                nc.sync.dma_start(
                    out=xstage[:, j, n, :],
                    in_=x_d[n, j * P:(j + 1) * P].rearrange(
                        "p h w -> p (h w)"),
                )
                interior = xs1[:, 2 * n + j, G:G + IMG].rearrange(
                    "p (r c) -> p r c", c=PW)[:, 1:1 + H, 1:1 + W]
                nc.scalar.sign(
                    interior,
                    xstage[:, j, n, :].rearrange("p (r c) -> p r c", c=W),
                )
            if n == 0:
                nc.sync.dma_start(out=w1s, in_=w1_d)

        w2s = big.tile([P, 9, J, C], F8)
        nc.sync.dma_start(out=w2s, in_=w2_d)
        gb_t = []
        for li, gb_d in enumerate((gb1_d, gb2_d)):
            per = []
            for j in range(J):
                g_t = small.tile([P, 1], F32, name=f"g{li}{j}",
                                 tag=f"g{li}{j}")
                b_t = small.tile([P, 1], F32, name=f"b{li}{j}",
                                 tag=f"b{li}{j}")
                nc.sync.dma_start(out=g_t,
                                  in_=gb_d[0, j].rearrange("(p o) -> p o",
                                                           o=1))
                nc.sync.dma_start(out=b_t,
                                  in_=gb_d[1, j].rearrange("(p o) -> p o",
                                                           o=1))
                per.append((g_t, b_t))
            gb_t.append(per)

        # ---- layer 1: conv block 0, AR under block 1's shadow, conv block 1
        c1raw = big.tile([P, J, NPC, HW], F16)
        c2raw = big.tile([P, J, NPC, HW], F16)
        sums10 = small.tile([P, 16], F32, tag="s10")
        sumsqs10 = small.tile([P, 16], F32, tag="q10")
        for n in range(NPC):
            for half in range(2):
                _chunk(nc, xs1, w1s, c1raw, 0, n, half, psum, scratch,
                       sums10, sumsqs10, sq_on_act=True)
        st1 = small.tile([P, 4], F32, tag="st1")
        nc.vector.reduce_sum(st1[:, 0:1], sums10, axis=mybir.AxisListType.X)
        nc.vector.reduce_sum(st1[:, 1:2], sumsqs10,
                             axis=mybir.AxisListType.X)

        sums11 = small.tile([P, 16], F32, tag="s11")
        sumsqs11 = small.tile([P, 16], F32, tag="q11")
        for n in range(NPC):
            for half in range(2):
                _chunk(nc, xs1, w1s, c1raw, 1, n, half, psum, scratch,
                       sums11, sumsqs11, sq_on_act=True)
        nc.vector.reduce_sum(st1[:, 2:3], sums11, axis=mybir.AxisListType.X)
        nc.vector.reduce_sum(st1[:, 3:4], sumsqs11,
                             axis=mybir.AxisListType.X)
        # one combined AR for both blocks: nothing can overlap conv1 anyway
        # (the dummy holds ncfw until ~conv1 end), so a second serialized
        # mesh would only add latency
        _ar_send(st1, "1")
        stg1 = _ar_recv("1", 4)
        sc1 = [None, None]
        for j in range(2):
            sc1[j] = _bn_coeffs(nc, small, stg1[:, 2 * j:2 * j + 1],
                                stg1[:, 2 * j + 1:2 * j + 2],
                                gb_t[0][j][0], gb_t[0][j][1], eps_t,
                                f"1{j}")

        # ---- conv2 block 0, interlayer sign interleaved image-by-image
        sums20 = small.tile([P, 16], F32, tag="s20")
        sumsqs20 = small.tile([P, 16], F32, tag="q20")
        for n in range(NPC):
            for j in range(2):
                interior = xs2[:, 2 * n + j, G:G + IMG].rearrange(
                    "p (r c) -> p r c", c=PW)[:, 1:1 + H, 1:1 + W]
                nc.scalar.activation(
                    interior,
                    c1raw[:, j, n, :].rearrange("p (r c) -> p r c", c=W),
                    mybir.ActivationFunctionType.Sign,
                    bias=sc1[j][1], scale=sc1[j][0],
                )
            for half in range(2):
                _chunk(nc, xs2, w2s, c2raw, 0, n, half, psum, scratch,
                       sums20, sumsqs20, sq_on_act=SQ20_ON_ACT)
        st20 = small.tile([P, 2], F32, tag="st20")
        nc.vector.reduce_sum(st20[:, 0:1], sums20, axis=mybir.AxisListType.X)
        nc.vector.reduce_sum(st20[:, 1:2], sumsqs20,
                             axis=mybir.AxisListType.X)
        _ar_send(st20, "20")

        # ---- conv2 block 1, with block-0 coeffs+outputs in its shadow
        sums21 = small.tile([P, 16], F32, tag="s21")
        sumsqs21 = small.tile([P, 16], F32, tag="q21")
        sc20 = None
        for n in range(NPC):
            for half in range(2):
                _chunk(nc, xs2, w2s, c2raw, 1, n, half, psum, scratch,
                       sums21, sumsqs21, sq_on_act=True)
                ci = 2 * n + half
                if ci == 5:
                    stg20 = _ar_recv("20", 2)
                    sc20 = _bn_coeffs(nc, small, stg20[:, 0:1], stg20[:, 1:2],
                                      gb_t[1][0][0], gb_t[1][0][1], eps_t,
                                      "20")
                if ci >= 7:
                    _emit_out(0, ci - 7, sc20[0], sc20[1], "act")
        st21 = small.tile([P, 2], F32, tag="st21")
        nc.vector.reduce_sum(st21[:, 0:1], sums21, axis=mybir.AxisListType.X)
        nc.vector.reduce_sum(st21[:, 1:2], sumsqs21,
                             axis=mybir.AxisListType.X)
        _ar_send(st21, "21")

        # leftover block-0 outputs run during the exchange
        for ci in range(9, 16):
            _emit_out(0, ci, sc20[0], sc20[1], "act")
        stg21 = _ar_recv("21", 2)
        sc21 = _bn_coeffs(nc, small, stg21[:, 0:1], stg21[:, 1:2],
                          gb_t[1][1][0], gb_t[1][1][1], eps_t, "21")
        for ci in range(16):
            _emit_out(1, ci, sc21[0], sc21[1], "dve" if ci < 2 else "act")

    nc.compile()
    return nc


def _pack_w(w):
    # [co, ci, kh, kw] -> sign -> [ci%128, kh*3+kw, ci//128, co] fp8e4
    s = np.sign(w.astype(np.float32)).reshape(C, J, P, 9)
    return np.ascontiguousarray(s.transpose(2, 3, 1, 0)).astype(
        ml_dtypes.float8_e4m3)


def _pack_gb(g, b):
    return np.ascontiguousarray(
        np.stack([g, b]).astype(np.float32).reshape(2, J, P))


def kernel(x, w1, g1, b1, w2, g2, b2, _profile=False):
    if "nc" not in _cache:
        _cache["nc"] = _build()
    nc = _cache["nc"]

    x = np.ascontiguousarray(x, np.float32)
    w1p, w2p = _pack_w(w1), _pack_w(w2)
    gb1, gb2 = _pack_gb(g1, b1), _pack_gb(g2, b2)
    in_maps = [
        {"x": x[c * NPC:(c + 1) * NPC], "w1p": w1p, "w2p": w2p,
         "gb1": gb1, "gb2": gb2}
        for c in range(N_CORES)
    ]
    res = bass_utils.run_bass_kernel_spmd(
        nc, in_maps, core_ids=list(range(N_CORES)), trace=_profile)
    y = np.concatenate([res.results[c]["y"] for c in range(N_CORES)], axis=0)
    if _profile:
        kernel.last_exec_time_ns = res.exec_time_ns
        kernel.last_results = res
    return y
